# revision 1
# baseline (speedup 1.0000x reference)
"""Trainium2 Bass kernel for the part-map heatmap-pyramid encoder.

Contract: kernel(part_maps, features) -> (64, 369952) float32.
Data parallel over batch: 8 samples per NeuronCore x 8 cores.

Per-core pipeline:
  1. moments:  mom[row, j] = sum_pix P[row,pix] * basis_j(pix)  (TensorE,
     fp32, accumulated over 32 pixel-chunks while the input streams in).
     Input is staged host-side transposed so no on-device transpose needed.
  2. tiny vector chain: mu/L_inv -> quadratic-form coeffs c0..c5 per row,
     with the heatmap's "+1" folded into c0.
  3. generation: proj = coeff^T @ [1,y,x,y^2,xy,x^2] as a rank-6 matmul per
     pyramid stage (TensorE), heat = 1/proj via one fused DVE reciprocal
     pass, streamed straight out to HBM in 0.5-1MB DMAs.
  4. stages 4-6 extras: part-sums via a 0/1 selection matmul, reciprocal,
     broadcast back via a replication matmul, elementwise normalize, and
     per-sample feature einsums as block-diagonal matmuls.
"""

import numpy as np

BN, NK, NF, HMAP = 64, 16, 64, 64
NCORES = 8
BL = BN // NCORES            # samples per core = 8
ROWS = BL * NK               # partition rows per core = 128
L_INV_SCAL = 0.8
EPS_DIST = 1e-6
EPS_COV = 1e-12

# (h, w, part_depth, (feat_slice_start, feat_slice_end))
STAGES = [(128, 128, NK, (0, 0)), (64, 64, NK, (0, 0)), (32, 32, NK, (0, 0)),
          (16, 16, NK, (4, NK)), (8, 8, 4, (2, 4)), (4, 4, 2, (0, 2))]
HWS = [h * w for (h, w, _, _) in STAGES]          # [16384,4096,1024,256,64,16]
GB_OFF = np.concatenate([[0], np.cumsum(HWS)])     # gen-basis col offsets
GB_TOT = int(GB_OFF[-1])                           # 21840

# per-sample output offsets
_off = 0
OUT_PH = []   # part_heat offset per stage
OUT_FM = []   # fmap offset per stage (or None)
for (h, w, pd, (s0, s1)) in STAGES:
    OUT_PH.append(_off)
    _off += pd * h * w
    if s1 - s0 != 0:
        OUT_FM.append(_off)
        _off += NF * h * w
    else:
        OUT_FM.append(None)
OUT_TOT = _off                                     # 369952

# generation matmul dtype: "float32r" (1 cyc/row) or "float32" (4 cyc/row)
GEN_DT_NAME = "float32r"


def _mesh_basis(h, w):
    """Per-pixel basis rows [1, y, x, y^2, x*y, x^2], pixel order i*w+j."""
    y = np.linspace(-1.0, 1.0, h, dtype=np.float64)
    x = np.linspace(-1.0, 1.0, w, dtype=np.float64)
    yy = np.repeat(y, w)
    xx = np.tile(x, h)
    return np.stack([np.ones_like(yy), yy, xx, yy * yy, yy * xx, xx * xx])


def _host_consts():
    # generation basis [6, GB_TOT]
    gb = np.concatenate([_mesh_basis(h, w) for (h, w, _, _) in STAGES],
                        axis=1).astype(np.float32)
    # moment basis, packed [128, 32*5]: mb[p, c*5+j] = basis_j(pixel c*128+p)
    bm = _mesh_basis(HMAP, HMAP)[1:6]              # [5, 4096] (drop the 1s row)
    mb = np.zeros((128, 32 * 5), dtype=np.float32)
    for c in range(32):
        mb[:, c * 5:(c + 1) * 5] = bm[:, c * 128:(c + 1) * 128].T
    ident = np.eye(128, dtype=np.float32)
    # selection matrices [128, 3*8]: sel[16b+k, si*8+b] = 1 if k in slice
    sel = np.zeros((128, 24), dtype=np.float32)
    # replication matrix [8, 128]: rep[b, 16b+k] = 1
    rep = np.zeros((8, 128), dtype=np.float32)
    for b in range(BL):
        for k in range(NK):
            rep[b, k * 8 + b] = 1.0
        for si, sidx in enumerate((3, 4, 5)):
            s0, s1 = STAGES[sidx][3]
            for k in range(s0, s1):
                sel[k * 8 + b, si * 8 + b] = 1.0
    return gb, mb, ident, sel, rep


def _host_wf(features_core):
    """Block-diagonal feature weights [128, 12*128].

    Block (si, g): W[16*b+k, 64*(b-2g)+n] = features[b, k, n] for
    b in {2g, 2g+1} and k in the stage's feature slice, else 0.
    """
    wf = np.zeros((128, 12 * 128), dtype=np.float32)
    for si, sidx in enumerate((3, 4, 5)):
        s0, s1 = STAGES[sidx][3]
        for g in range(4):
            blk = (si * 4 + g) * 128
            for bo in range(2):
                b = 2 * g + bo
                for k in range(s0, s1):
                    wf[k * 8 + b, blk + 64 * bo:blk + 64 * (bo + 1)] = \
                        features_core[b, k, :]
    return wf


_NC_CACHE = {}


def _build(gen_dt_name):
    import concourse.bass as bass
    import concourse.bacc as bacc
    import concourse.tile as tile
    from concourse import mybir

    f32 = mybir.dt.float32
    gen_dt = getattr(mybir.dt, gen_dt_name)
    AT = mybir.AluOpType

    nc = bacc.Bacc("TRN2", target_bir_lowering=False, debug=False)
    pt = nc.declare_dram_parameter("pt", [HMAP * HMAP, ROWS], f32, isOutput=False)
    gb1 = nc.declare_dram_parameter("gb1", [6, HWS[0]], gen_dt, isOutput=False)
    gbr = nc.declare_dram_parameter("gbr", [6, GB_TOT - HWS[0]], gen_dt,
                                    isOutput=False)
    mb = nc.declare_dram_parameter("mb", [128, 160], f32, isOutput=False)
    ident = nc.declare_dram_parameter("ident", [128, 128], f32, isOutput=False)
    sel = nc.declare_dram_parameter("sel", [128, 24], f32, isOutput=False)
    rep = nc.declare_dram_parameter("rep", [8, 128], f32, isOutput=False)
    wf = nc.declare_dram_parameter("wf", [128, 12 * 128], f32, isOutput=False)
    out = nc.declare_dram_parameter("out", [BL, OUT_TOT], f32, isOutput=True)

    with tile.TileContext(nc) as tc:
        import contextlib
        ctx = contextlib.ExitStack()
        with ctx:
            consts = ctx.enter_context(tc.tile_pool(name="consts", bufs=1))
            ptp = ctx.enter_context(tc.tile_pool(name="ptp", bufs=8))
            gbp = ctx.enter_context(tc.tile_pool(name="gbp", bufs=2))
            sm = ctx.enter_context(tc.tile_pool(name="sm", bufs=1))
            hp = ctx.enter_context(tc.tile_pool(name="hp", bufs=6))
            sp = ctx.enter_context(tc.tile_pool(name="sp", bufs=3))
            pgen = ctx.enter_context(tc.tile_pool(name="pgen", bufs=4, space="PSUM"))
            pmisc = ctx.enter_context(tc.tile_pool(name="pmisc", bufs=2, space="PSUM"))
            pfm = ctx.enter_context(tc.tile_pool(name="pfm", bufs=2, space="PSUM"))

            # ---- constants in ----
            from concourse.tile import add_dep_helper


            smb = consts.tile([128, 160], f32)
            d_mb = nc.sync.dma_start(out=smb, in_=mb[:, :])
            sident = consts.tile([128, 128], f32)
            d_id = nc.sync.dma_start(out=sident, in_=ident[:, :])

            # ---- phase 1: moments (exact fp32) ----
            psmom = pmisc.tile([128, 8], f32, tag="pmisc")
            for c in range(8):
                ptc = ptp.tile([128, 4, 128], f32, tag="ptc")
                nc.sync.dma_start(
                    out=ptc,
                    in_=pt[c * 512:(c + 1) * 512, :].rearrange(
                        "(i p) r -> p i r", p=128),
                )
                for i in range(4):
                    cc = c * 4 + i
                    mm = nc.tensor.matmul(
                        psmom[:, 0:5],
                        lhsT=ptc[:, i, :],
                        rhs=smb[:, cc * 5:(cc + 1) * 5],
                        start=(cc == 0),
                        stop=(cc == 31),
                    )



            # ---- phase 2: per-row coefficients ----
            def t(cols, tag):
                return sm.tile([128, cols], f32, tag=tag, name=tag)

            epsc = t(1, "epsc")
            nc.vector.memset(epsc, EPS_COV)
            u = t(3, "u"); v = t(3, "v")
            nc.vector.tensor_copy(out=u[:, 0:1], in_=psmom[:, 0:1])
            nc.vector.tensor_copy(out=u[:, 1:3], in_=psmom[:, 0:2])
            nc.vector.tensor_copy(out=v[:, 0:2], in_=psmom[:, 0:2])
            nc.vector.tensor_copy(out=v[:, 2:3], in_=psmom[:, 1:2])
            prod = t(3, "prod")
            nc.vector.tensor_tensor(out=prod, in0=u, in1=v, op=AT.mult)
            cov = t(3, "cov")
            nc.vector.tensor_tensor(out=cov, in0=psmom[:, 2:5], in1=prod,
                                    op=AT.subtract)
            a = t(1, "a")
            nc.scalar.activation(out=a, in_=cov[:, 0:1],
                                 func=mybir.ActivationFunctionType.Sqrt,
                                 bias=epsc)
            az = t(1, "az")
            nc.vector.tensor_scalar_add(out=az, in0=a, scalar1=EPS_COV)
            ainv = t(1, "ainv")
            nc.vector.reciprocal_approx_fast(out=ainv, in_=az)
            b = t(1, "b")
            nc.vector.tensor_tensor(out=b, in0=cov[:, 1:2], in1=ainv, op=AT.mult)
            b2 = t(1, "b2")
            nc.vector.tensor_tensor(out=b2, in0=b, in1=b, op=AT.mult)
            t2 = t(1, "t2")
            nc.vector.tensor_tensor(out=t2, in0=cov[:, 2:3], in1=b2,
                                    op=AT.subtract)
            cc_ = t(1, "cc_")
            nc.scalar.activation(out=cc_, in_=t2,
                                 func=mybir.ActivationFunctionType.Sqrt,
                                 bias=epsc)
            det = t(1, "det")
            nc.vector.tensor_tensor(out=det, in0=a, in1=cc_, op=AT.mult)
            dz = t(1, "dz")
            nc.vector.tensor_scalar_add(out=dz, in0=det, scalar1=EPS_COV)
            spr = t(1, "spr")
            nc.vector.reciprocal_approx_fast(out=spr, in_=dz)
            s2 = t(1, "s2")
            nc.vector.tensor_tensor(out=s2, in0=spr, in1=spr, op=AT.mult)
            q = t(1, "q")
            nc.vector.tensor_scalar_mul(out=q, in0=s2,
                                        scalar1=L_INV_SCAL * L_INV_SCAL)
            c2s = t(1, "c2s")
            nc.vector.tensor_tensor(out=c2s, in0=cc_, in1=cc_, op=AT.mult)
            bc2 = t(1, "bc2")
            nc.vector.tensor_tensor(out=bc2, in0=b2, in1=c2s, op=AT.add)

            coef = sm.tile([128, 6], f32, tag="coef")
            # A = q*(b^2+c^2), B = -2*q*a*b, C = q*a^2
            nc.vector.tensor_tensor(out=coef[:, 3:4], in0=q, in1=bc2, op=AT.mult)
            ab = t(1, "ab")
            nc.vector.tensor_tensor(out=ab, in0=a, in1=b, op=AT.mult)
            nc.vector.scalar_tensor_tensor(out=coef[:, 4:5], in0=ab, scalar=-2.0,
                                           in1=q, op0=AT.mult, op1=AT.mult)
            a2 = t(1, "a2")
            nc.vector.tensor_tensor(out=a2, in0=a, in1=a, op=AT.mult)
            nc.vector.tensor_tensor(out=coef[:, 5:6], in0=q, in1=a2, op=AT.mult)
            # py = eps - mu_y, px = eps - mu_x
            pp = t(2, "pp")
            nc.vector.tensor_scalar(out=pp, in0=psmom[:, 0:2], scalar1=-1.0,
                                    scalar2=EPS_DIST, op0=AT.mult, op1=AT.add)
            u2 = t(3, "u2"); v2 = t(3, "v2")
            nc.vector.tensor_copy(out=u2[:, 0:1], in_=pp[:, 0:1])
            nc.vector.tensor_copy(out=u2[:, 1:3], in_=pp)
            nc.vector.tensor_copy(out=v2[:, 0:2], in_=pp)
            nc.vector.tensor_copy(out=v2[:, 2:3], in_=pp[:, 1:2])
            pyx = t(3, "pyx")
            nc.vector.tensor_tensor(out=pyx, in0=u2, in1=v2, op=AT.mult)
            terms = t(3, "terms")
            nc.vector.tensor_tensor(out=terms, in0=coef[:, 3:6], in1=pyx,
                                    op=AT.mult)
            c0s = t(1, "c0s")
            nc.vector.reduce_sum(out=c0s, in_=terms, axis=mybir.AxisListType.X)
            # fold heat's +1 into the constant coefficient
            nc.vector.tensor_scalar_add(out=coef[:, 0:1], in0=c0s, scalar1=1.0)
            t4 = t(1, "t4"); t5 = t(1, "t5")
            nc.vector.tensor_tensor(out=t4, in0=coef[:, 3:4], in1=pp[:, 0:1],
                                    op=AT.mult)
            nc.vector.tensor_tensor(out=t5, in0=coef[:, 4:5], in1=pp[:, 1:2],
                                    op=AT.mult)
            nc.vector.scalar_tensor_tensor(out=coef[:, 1:2], in0=t4, scalar=2.0,
                                           in1=t5, op0=AT.mult, op1=AT.add)
            t6 = t(1, "t6"); t7 = t(1, "t7")
            nc.vector.tensor_tensor(out=t6, in0=coef[:, 4:5], in1=pp[:, 0:1],
                                    op=AT.mult)
            nc.vector.tensor_tensor(out=t7, in0=coef[:, 5:6], in1=pp[:, 1:2],
                                    op=AT.mult)
            nc.vector.scalar_tensor_tensor(out=coef[:, 2:3], in0=t7, scalar=2.0,
                                           in1=t6, op0=AT.mult, op1=AT.add)

            # transpose coeffs -> [6, 128]
            pst = pmisc.tile([6, 128], f32, tag="pmisc")
            nc.tensor.transpose(pst, coef, sident)
            coefT = sm.tile([6, 128], gen_dt, tag="coefT")
            nc.vector.tensor_copy(out=coefT, in_=pst)

            # ---- phase 3: heat generation ----
            def gen_heat(basis, b0, n, dst, dst_col):
                """proj matmul + reciprocal for basis cols [b0, b0+n),
                writing heat into dst[:, dst_col:dst_col+n]."""
                for m0 in range(0, n, 512):
                    mn = min(512, n - m0)
                    ps = pgen.tile([128, mn], f32, tag="ps")
                    nc.tensor.matmul(
                        ps, lhsT=coefT, rhs=basis[:, b0 + m0:b0 + m0 + mn],
                        start=True, stop=True)
                    nc.vector.reciprocal_approx_fast(
                        out=dst[:, dst_col + m0:dst_col + m0 + mn], in_=ps)

            # Output emitter: split a column slice into two half-partition
            # DMAs on rotating rings (SP weighted low - it carries inputs).
            _ring_pat = (nc.gpsimd, nc.sync, nc.scalar)
            _ring_n = [0]

            def emit_out(dview, ht, dcol, scol, width):
                eng = _ring_pat[_ring_n[0] % len(_ring_pat)]
                _ring_n[0] += 1
                eng.dma_start(out=dview[:, :, dcol:dcol + width],
                              in_=ht[:, scol:scol + width])

            # stage 0: stream basis chunks in, heat straight out
            st1 = out[:, OUT_PH[0]:OUT_PH[0] + NK * HWS[0]].rearrange(
                "b (k f) -> k b f", k=NK)
            for dc in range(4):
                gbc = gbp.tile([6, 4096], gen_dt, name="gbc")
                geng = nc.scalar if dc < 2 else nc.sync
                geng.dma_start(out=gbc, in_=gb1[:, dc * 4096:(dc + 1) * 4096])
                for half in range(2):
                    n0 = dc * 4096 + half * 2048
                    ht = hp.tile([128, 2048], f32, tag="ht")
                    gen_heat(gbc, half * 2048, 2048, ht, 0)
                    for q in range(4):
                        emit_out(st1, ht, n0 + q * 512, q * 512, 512)

            # late-needed constants (stage >= 2): loaded during stage-1 streaming
            sgbr = consts.tile([6, GB_TOT - HWS[0]], gen_dt)
            gw = GB_TOT - HWS[0]
            g3 = gw // 4
            nc.sync.dma_start(out=sgbr[:, 0:g3], in_=gbr[:, 0:g3])
            nc.scalar.dma_start(out=sgbr[:, g3:2 * g3], in_=gbr[:, g3:2 * g3])
            nc.gpsimd.dma_start(out=sgbr[:, 2 * g3:gw], in_=gbr[:, 2 * g3:gw])
            ssel = consts.tile([128, 24], f32)
            d_sel = nc.sync.dma_start(out=ssel, in_=sel[:, :])
            srep = consts.tile([8, 128], f32)
            d_rep = nc.sync.dma_start(out=srep, in_=rep[:, :])
            swf = consts.tile([128, 12 * 128], f32)
            d_wf = nc.sync.dma_start(out=swf, in_=wf[:, :])

            # stages 1-2: resident basis, stream straight out
            for sidx, dma_cols in ((1, 2048), (2, 1024)):
                hw = HWS[sidx]
                goff = int(GB_OFF[sidx]) - HWS[0]
                stv = out[:, OUT_PH[sidx]:OUT_PH[sidx] + NK * hw].rearrange(
                    "b (k f) -> k b f", k=NK)
                for ci, n0 in enumerate(range(0, hw, dma_cols)):
                    ht = hp.tile([128, dma_cols], f32, tag="ht")
                    gen_heat(sgbr, goff + n0, dma_cols, ht, 0)
                    h4 = dma_cols // 4
                    for q in range(4):
                        emit_out(stv, ht, n0 + q * h4, q * h4, h4)

            # stages 3-5: heat tiles stay in SBUF
            H = {}
            for sidx in (3, 4, 5):
                hw = HWS[sidx]
                Hs = sp.tile([128, hw], f32, tag=f"H{sidx}", bufs=1)
                gen_heat(sgbr, int(GB_OFF[sidx]) - HWS[0], hw, Hs, 0)
                H[sidx] = Hs

            # part_heat outputs
            for sidx in (3, 4, 5):
                hw = HWS[sidx]
                pd = STAGES[sidx][2]
                stv = out[:, OUT_PH[sidx]:OUT_PH[sidx] + pd * hw].rearrange(
                    "b (k f) -> k b f", k=pd)
                eng = nc.scalar if sidx % 2 else nc.gpsimd
                eng.dma_start(out=stv, in_=H[sidx][0:pd * BL, :])

            # fmap chains
            for si, sidx in enumerate((3, 4, 5)):
                hw = HWS[sidx]
                pss = pmisc.tile([8, hw], f32, tag="pmisc")
                nc.tensor.matmul(pss, lhsT=ssel[:, si * 8:(si + 1) * 8],
                                 rhs=H[sidx], start=True, stop=True)
                rt = sp.tile([8, hw], f32, tag="rt", bufs=2)
                nc.vector.tensor_scalar_add(out=rt, in0=pss, scalar1=1.0)
                rr = sp.tile([8, hw], f32, tag="rr", bufs=2)
                nc.vector.reciprocal_approx_fast(out=rr, in_=rt)
                psR = pmisc.tile([128, hw], f32, tag="pmisc")
                nc.tensor.matmul(psR, lhsT=srep, rhs=rr, start=True, stop=True)
                Hn = sp.tile([128, hw], f32, tag="Hn", bufs=2)
                nc.vector.tensor_tensor(out=Hn, in0=H[sidx], in1=psR, op=AT.mult)
                stf = out[:, OUT_FM[sidx]:OUT_FM[sidx] + NF * hw].rearrange(
                    "b (n f) -> b n f", n=NF)
                for g in range(4):
                    psF = pfm.tile([128, hw], f32, tag="pfm")
                    nc.tensor.matmul(
                        psF, lhsT=swf[:, (si * 4 + g) * 128:(si * 4 + g + 1) * 128],
                        rhs=Hn, start=True, stop=True)
                    fm = sp.tile([128, hw], f32, tag="fm", bufs=12)
                    nc.vector.tensor_copy(out=fm, in_=psF)
                    nc.scalar.dma_start(out=stf[2 * g], in_=fm[0:64, :])
                    nc.gpsimd.dma_start(out=stf[2 * g + 1], in_=fm[64:128, :])
    nc.compile()
    return nc


def _get_nc():
    if GEN_DT_NAME not in _NC_CACHE:
        _NC_CACHE[GEN_DT_NAME] = _build(GEN_DT_NAME)
    return _NC_CACHE[GEN_DT_NAME]


def _in_maps(part_maps, features):
    part_maps = np.asarray(part_maps, dtype=np.float32)
    features = np.asarray(features, dtype=np.float32)
    gb, mb, ident, sel, rep = _host_consts()
    gb1c = np.ascontiguousarray(gb[:, :HWS[0]])
    gbrc = np.ascontiguousarray(gb[:, HWS[0]:])
    in_maps = []
    for core in range(NCORES):
        pm = part_maps[core * BL:(core + 1) * BL]          # [8, 16, 64, 64]
        # k-major row order: row r = k*8 + b
        pt = np.ascontiguousarray(
            pm.transpose(1, 0, 2, 3).reshape(ROWS, HMAP * HMAP).T)  # [4096,128]
        wf = _host_wf(features[core * BL:(core + 1) * BL])
        in_maps.append({"pt": pt, "gb1": gb1c, "gbr": gbrc, "mb": mb,
                        "ident": ident, "sel": sel, "rep": rep, "wf": wf})
    return in_maps


def _run(part_maps, features, trace=False):
    from concourse.bass_utils import run_bass_kernel_spmd
    nc = _get_nc()
    res = run_bass_kernel_spmd(nc, _in_maps(part_maps, features),
                               list(range(NCORES)), trace=trace)
    outs = [res.results[i]["out"] for i in range(NCORES)]
    return np.concatenate(outs, axis=0), res


def kernel(part_maps, features):
    out, _ = _run(part_maps, features, trace=False)
    return out



# revision 7
# speedup vs baseline: 4.4712x; 4.4712x over previous
"""Trainium2 Bass kernel v4 for the part-map heatmap-pyramid encoder.

Contract: kernel(part_maps, features) -> (64, 369952) float32.
Data parallel over batch: 8 samples per NeuronCore x 8 cores.

Per-core pipeline:
  1. moments: 32 accumulating fp32 matmuls over pixel chunks -> [128, 5];
     moment basis is folded into the same DRAM load as the part maps.
  2. coefficient chain split over Pool+DVE (no sqrt: L_inv^T L_inv =
     (0.8/det)^2 [[s11,-s01],[-s01,s00]]), one divide.
  3. transpose coef -> [6, 128] bf16.
  4. gen: rank-6 bf16 matmuls (PE) from a resident [6, 21840] bf16 basis
     into [128, 1024] PSUM tiles; heat = 1/proj via Pool divide / DVE
     reciprocal_approx_fast; all heat in one [128, 21840] f32 SBUF tile.
  5. fmap chains for stages 3-5, interleaved into the gen stream.
  6. stores: padded flat DRAM regions (500ns each in-model), host unpack.
"""

import numpy as np

BN, NK, NF, HMAP = 64, 16, 64, 64
NCORES = 8
BL = BN // NCORES            # samples per core = 8
ROWS = BL * NK               # partition rows per core = 128
L_INV_SCAL = 0.8
EPS_DIST = 1e-6
EPS_COV = 1e-12

STAGES = [(128, 128, NK, (0, 0)), (64, 64, NK, (0, 0)), (32, 32, NK, (0, 0)),
          (16, 16, NK, (4, NK)), (8, 8, 4, (2, 4)), (4, 4, 2, (0, 2))]
HWS = [h * w for (h, w, _, _) in STAGES]
HOFF = np.concatenate([[0], np.cumsum(HWS)]).astype(int)
GB_TOT = int(HOFF[-1])       # 21840

_off = 0
OUT_PH = []
OUT_FM = []
for (h, w, pd, (s0, s1)) in STAGES:
    OUT_PH.append(_off)
    _off += pd * h * w
    if s1 - s0 != 0:
        OUT_FM.append(_off)
        _off += NF * h * w
    else:
        OUT_FM.append(None)
OUT_TOT = _off               # 369952

# ---- padded output regions: [elems/64 rows, 80] each (64 data cols) ----
STP = 80
# R0, R1, R2 (stage 0-2 part heats), R345 (stages 3-5 cols incl garbage
# rows), F3, F4, F5 (fmap tiles)
REGION_ELEMS = [128 * HWS[0], 128 * HWS[1], 128 * HWS[2], 128 * 336,
                128 * 4 * HWS[3], 128 * 4 * HWS[4], 128 * 4 * HWS[5]]
REGION_W = [64, 64, 64, 16, 64, 64, 64]
REGION_ROWS = [e // w for e, w in zip(REGION_ELEMS, REGION_W)]
REGION_ROFF = np.concatenate([[0], np.cumsum(REGION_ROWS)]).astype(int)
OUT_ROWS = int(REGION_ROFF[-1])

# 2-band basis split point (heat cols >= BSPLIT live in sgb12 rows 0-5)
BSPLIT = 10920
# gen tiles: (heat_offset, ncols, band): band A ordered stage345 tail,
# stage2, stage1, stage0-back; then band B = stage0 front
GEN_TILES = [(int(HOFF[3]), 336, 0)]
for c in range(int(HOFF[2]), int(HOFF[3]), 512):
    GEN_TILES.append((c, 512, 0))
for c in range(int(HOFF[1]), int(HOFF[2]), 512):
    GEN_TILES.append((c, 512, 0))
for c in range(BSPLIT, int(HOFF[1]), 512):
    GEN_TILES.append((c, min(512, int(HOFF[1]) - c), 0))
for c in range(0, BSPLIT, 512):
    GEN_TILES.append((c, min(512, BSPLIT - c), 1))


def _mesh_basis(h, w):
    y = np.linspace(-1.0, 1.0, h, dtype=np.float64)
    x = np.linspace(-1.0, 1.0, w, dtype=np.float64)
    yy = np.repeat(y, w)
    xx = np.tile(x, h)
    return np.stack([np.ones_like(yy), yy, xx, yy * yy, yy * xx, xx * xx])


def _bf16(a):
    import ml_dtypes
    return np.asarray(a, dtype=np.float32).astype(ml_dtypes.bfloat16)


def _host_consts():
    gb = np.concatenate([_mesh_basis(h, w) for (h, w, _, _) in STAGES], axis=1)
    # 2-band layout [12, BSPLIT]: rows 0-5 = cols BSPLIT.., rows 6-11 = front
    gb12 = np.concatenate([gb[:, BSPLIT:], gb[:, 0:BSPLIT]], axis=0)
    bm = _mesh_basis(HMAP, HMAP)[1:6]              # [5, 4096]
    mb = np.zeros((128, 160), dtype=np.float32)
    for c in range(32):
        mb[:, c * 5:(c + 1) * 5] = bm[:, c * 128:(c + 1) * 128].T
    cst = np.eye(128, dtype=np.float32)
    sel = np.zeros((128, 24), dtype=np.float32)
    rep = np.zeros((128, 128), dtype=np.float32)
    for b in range(BL):
        for k in range(NK):
            rep[b, k * 8 + b] = 1.0
        for si, sidx in enumerate((3, 4, 5)):
            s0, s1 = STAGES[sidx][3]
            for k in range(s0, s1):
                sel[k * 8 + b, si * 8 + b] = 1.0
    return _bf16(gb12), mb, cst, _bf16(sel), _bf16(rep)


def _host_wf(features_core):
    wf = np.zeros((128, 12 * 128), dtype=np.float32)
    for si, sidx in enumerate((3, 4, 5)):
        s0, s1 = STAGES[sidx][3]
        for g in range(4):
            blk = (si * 4 + g) * 128
            for bo in range(2):
                b = 2 * g + bo
                for k in range(s0, s1):
                    wf[k * 8 + b, blk + 64 * bo:blk + 64 * (bo + 1)] = \
                        features_core[b, k, :]
    return _bf16(wf)


_NC_CACHE = {}


def _build():
    import concourse.bass as bass
    import concourse.bacc as bacc
    import concourse.tile as tile
    from concourse import mybir

    f32 = mybir.dt.float32
    bf16 = mybir.dt.bfloat16
    AT = mybir.AluOpType

    nc = bacc.Bacc("TRN2", target_bir_lowering=False, debug=False)
    # pta: [mb(160) | pt(4096)] f32
    pta = nc.declare_dram_parameter("pta", [128, 4256], f32, isOutput=False)
    gbb = nc.declare_dram_parameter("gbb", [12, BSPLIT], bf16, isOutput=False)
    cstb = nc.declare_dram_parameter("cstb", [128, 128], f32, isOutput=False)
    wfb = nc.declare_dram_parameter("wfb", [128, 1688], bf16, isOutput=False)
    outp = nc.declare_dram_parameter("out", [OUT_ROWS, STP], f32,
                                     isOutput=True)

    def oreg(ri):
        r0, r1 = int(REGION_ROFF[ri]), int(REGION_ROFF[ri + 1])
        return outp[r0:r1, 0:REGION_W[ri]]

    with tile.TileContext(nc) as tc:
        import contextlib
        ctx = contextlib.ExitStack()
        with ctx:
            sb = ctx.enter_context(tc.tile_pool(name="sb", bufs=1))
            pgen = ctx.enter_context(tc.tile_pool(name="pgen", bufs=6,
                                                  space="PSUM"))
            pmisc = ctx.enter_context(tc.tile_pool(name="pmisc", bufs=1,
                                                   space="PSUM"))

            spta = sb.tile([128, 4256], f32, tag="spta")
            scst = sb.tile([128, 128], f32, tag="scst")
            sgb = sb.tile([12, BSPLIT], bf16, tag="sgb")
            swf = sb.tile([128, 1688], bf16, tag="swf")

            # pt pieces: [0:1184]=mb+chunks0-7, then 1024-col pieces
            nc.sync.dma_start(out=spta[:, 0:1184], in_=pta[:, 0:1184])
            nc.scalar.dma_start(out=spta[:, 1184:2208], in_=pta[:, 1184:2208])
            nc.gpsimd.dma_start(out=spta[:, 2208:3232], in_=pta[:, 2208:3232])
            nc.gpsimd.dma_start(out=spta[:, 3232:4256], in_=pta[:, 3232:4256])
            # basis on SP in gen-consumption order (high cols first);
            # Act only gets small early loads so its recip window is clear
            nc.scalar.dma_start(out=scst, in_=cstb[:, :])
            nc.sync.dma_start(out=sgb[:, 9216:BSPLIT],
                              in_=gbb[:, 9216:BSPLIT])
            nc.scalar.dma_start(out=swf[:, 0:664], in_=wfb[:, 0:664])
            nc.scalar.dma_start(out=swf[:, 664:1688], in_=wfb[:, 664:1688])
            nc.scalar.dma_start(out=sgb[:, 5460:9216], in_=gbb[:, 5460:9216])
            nc.sync.dma_start(out=sgb[:, 0:5460], in_=gbb[:, 0:5460])

            czzA = sb.tile([12, 128], bf16, tag="czzA")
            nc.gpsimd.memset(czzA, 0.0)
            czzB = sb.tile([12, 128], bf16, tag="czzB")
            nc.gpsimd.memset(czzB, 0.0)

            # ---- moments (with PE pstate-warming fillers) ----
            def warm(k):
                for _ in range(k):
                    pw = pgen.tile([128, 64], f32, tag="ps", name="pw")
                    nc.tensor.matmul(pw, lhsT=spta[:, 160:288],
                                     rhs=spta[:, 0:64], start=True, stop=True)

            psmom = pmisc.tile([128, 8], f32, tag="m1")
            for n_, c in enumerate(range(32)):
                nc.tensor.matmul(
                    psmom[:, 0:5],
                    lhsT=spta[:, 160 + c * 128:160 + (c + 1) * 128],
                    rhs=spta[:, c * 5:(c + 1) * 5],
                    start=(n_ == 0),
                    stop=(n_ == 31),
                )
                if c in (7, 15, 23):
                    warm(2)
            warm(8)

            # ---- coefficient chain, Pool (g) / DVE (v) split ----
            def t(cols, tag):
                return sb.tile([128, cols], f32, tag=tag, name=tag)

            v = nc.vector
            g = v
            mom = t(5, "mom")
            v.tensor_copy(out=mom, in_=psmom[:, 0:5])  # muy mux Eyy Eyx Exx
            pyy = t(1, "pyy"); pyx_ = t(1, "pyx_"); pxx = t(1, "pxx")
            g.tensor_tensor(out=pyy, in0=mom[:, 0:1], in1=mom[:, 0:1],
                            op=AT.mult)
            g.tensor_tensor(out=pyx_, in0=mom[:, 0:1], in1=mom[:, 1:2],
                            op=AT.mult)
            g.tensor_tensor(out=pxx, in0=mom[:, 1:2], in1=mom[:, 1:2],
                            op=AT.mult)
            c00 = t(1, "c00"); c01 = t(1, "c01"); c11 = t(1, "c11")
            g.tensor_tensor(out=c00, in0=mom[:, 2:3], in1=pyy, op=AT.subtract)
            g.tensor_tensor(out=c01, in0=mom[:, 3:4], in1=pyx_,
                            op=AT.subtract)
            g.tensor_tensor(out=c11, in0=mom[:, 4:5], in1=pxx, op=AT.subtract)
            pp = t(2, "pp")
            v.tensor_scalar(out=pp, in0=mom[:, 0:2], scalar1=-1.0,
                            scalar2=EPS_DIST, op0=AT.mult, op1=AT.add)
            qyy = t(1, "qyy"); qyx = t(1, "qyx"); qxx = t(1, "qxx")
            v.tensor_tensor(out=qyy, in0=pp[:, 0:1], in1=pp[:, 0:1],
                            op=AT.mult)
            v.tensor_tensor(out=qyx, in0=pp[:, 0:1], in1=pp[:, 1:2],
                            op=AT.mult)
            v.tensor_tensor(out=qxx, in0=pp[:, 1:2], in1=pp[:, 1:2],
                            op=AT.mult)
            m1_ = t(1, "m1_"); m2_ = t(1, "m2_")
            g.tensor_tensor(out=m1_, in0=c00, in1=c11, op=AT.mult)
            g.tensor_tensor(out=m2_, in0=c01, in1=c01, op=AT.mult)
            d2 = t(1, "d2")
            g.tensor_tensor(out=d2, in0=m1_, in1=m2_, op=AT.subtract)
            d2s = t(1, "d2s")
            g.tensor_scalar(out=d2s, in0=d2,
                            scalar1=1.0 / (L_INV_SCAL * L_INV_SCAL),
                            scalar2=1e-20, op0=AT.mult, op1=AT.add)
            q = t(1, "q")
            v.reciprocal_approx_fast(out=q, in_=d2s)

            coef = sb.tile([128, 6], f32, tag="coef")
            g.tensor_tensor(out=coef[:, 3:4], in0=q, in1=c11, op=AT.mult)
            g.scalar_tensor_tensor(out=coef[:, 4:5], in0=q, scalar=-2.0,
                                   in1=c01, op0=AT.mult, op1=AT.mult)
            g.tensor_tensor(out=coef[:, 5:6], in0=q, in1=c00, op=AT.mult)
            tA = t(1, "tA"); tB = t(1, "tB"); tC = t(1, "tC")
            v.tensor_tensor(out=tA, in0=coef[:, 3:4], in1=qyy, op=AT.mult)
            v.tensor_tensor(out=tB, in0=coef[:, 4:5], in1=qyx, op=AT.mult)
            v.tensor_tensor(out=tC, in0=coef[:, 5:6], in1=qxx, op=AT.mult)
            c0a = t(1, "c0a")
            v.tensor_tensor(out=c0a, in0=tA, in1=tB, op=AT.add)
            c0b = t(1, "c0b")
            v.tensor_tensor(out=c0b, in0=c0a, in1=tC, op=AT.add)
            v.tensor_scalar_add(out=coef[:, 0:1], in0=c0b, scalar1=1.0)
            t4 = t(1, "t4"); t5 = t(1, "t5")
            g.tensor_tensor(out=t4, in0=coef[:, 3:4], in1=pp[:, 0:1],
                            op=AT.mult)
            g.tensor_tensor(out=t5, in0=coef[:, 4:5], in1=pp[:, 1:2],
                            op=AT.mult)
            g.scalar_tensor_tensor(out=coef[:, 1:2], in0=t4, scalar=2.0,
                                   in1=t5, op0=AT.mult, op1=AT.add)
            t6 = t(1, "t6"); t7 = t(1, "t7")
            g.tensor_tensor(out=t6, in0=coef[:, 4:5], in1=pp[:, 0:1],
                            op=AT.mult)
            g.tensor_tensor(out=t7, in0=coef[:, 5:6], in1=pp[:, 1:2],
                            op=AT.mult)
            g.scalar_tensor_tensor(out=coef[:, 2:3], in0=t7, scalar=2.0,
                                   in1=t6, op0=AT.mult, op1=AT.add)

            pst = pmisc.tile([6, 128], f32, tag="m1")
            nc.tensor.transpose(pst, coef, scst[:, 0:128])
            coefT = sb.tile([6, 128], bf16, tag="coefT")
            nc.vector.tensor_copy(out=coefT, in_=pst)
            # band lhsT variants: A = [coefT; 0] (legal partition-32 copy),
            # B = [0; coefT] (SBUF->SBUF DMA; latency hides under band A)
            nc.vector.tensor_copy(out=czzA[0:6, :], in_=coefT)
            nc.sync.dma_start(out=czzB[6:12, :], in_=coefT)

            # ---- heat generation ----
            heat = sb.tile([128, GB_TOT], f32, tag="heat")

            def recip_dve(off, n, ps):
                nc.vector.reciprocal_approx_fast(
                    out=heat[:, off:off + n], in_=ps)

            def recip_act(off, n, ps):
                se = nc.scalar
                se.add_instruction(
                    mybir.InstActivation(
                        name=nc.get_next_instruction_name(),
                        func=mybir.ActivationFunctionType.Reciprocal,
                        ins=[se.lower_ap(ps),
                             mybir.ImmediateValue(dtype=f32, value=0.0),
                             mybir.ImmediateValue(dtype=f32, value=1.0),
                             mybir.ImmediateValue(dtype=f32, value=0.0)],
                        outs=[se.lower_ap(heat[:, off:off + n])],
                    ))

            # ---- fmap chains (bf16), fed from an Hb bf16 copy ----
            fstate = {}
            HB0 = int(HOFF[3])

            def fm_sel(si):
                hw = HWS[3 + si]
                hb0 = int(HOFF[3 + si]) - HB0
                pss = pmisc.tile([8, 256], f32, tag="m1")
                nc.tensor.matmul(pss[:, 0:hw],
                                 lhsT=swf[:, si * 8:si * 8 + 8],
                                 rhs=fstate["Hb"][:, hb0:hb0 + hw],
                                 start=True, stop=True)
                rt = sb.tile([8, 256], f32, tag="rt", bufs=2)
                nc.vector.tensor_scalar_add(out=rt[:, 0:hw], in0=pss[:, 0:hw],
                                            scalar1=1.0)
                rr = sb.tile([8, 256], f32, tag="rr", bufs=2)
                nc.vector.reciprocal_approx_fast(out=rr[:, 0:hw],
                                                 in_=rt[:, 0:hw])
                rrb = sb.tile([8, 256], bf16, tag="rrb", bufs=2)
                nc.vector.tensor_copy(out=rrb[:, 0:hw], in_=rr[:, 0:hw])
                fstate[si] = (rrb, hw)

            def fm_rep(si):
                rrb, hw = fstate[si]
                H = heat[:, int(HOFF[3 + si]):int(HOFF[3 + si]) + hw]
                psR = pmisc.tile([128, 256], f32, tag="m1")
                nc.tensor.matmul(psR[:, 0:hw], lhsT=swf[0:8, 24:152],
                                 rhs=rrb[:, 0:hw], start=True, stop=True)
                Hn = sb.tile([128, 256], bf16, tag="Hn", bufs=2)
                nc.vector.tensor_tensor(out=Hn[:, 0:hw], in0=H,
                                        in1=psR[:, 0:hw], op=AT.mult)
                fstate[si] = (Hn, hw)

            def fm_mm(si):
                Hn, hw = fstate[si]
                fm = sb.tile([128, 4 * hw], f32, tag=f"fm{si}")
                ntl = 2 if si == 0 else 1
                for tl in range(ntl):
                    psF = pmisc.tile([128, 512], f32, tag="psF")
                    for gi in range(4 // ntl):
                        gg = tl * (4 // ntl) + gi
                        nc.tensor.matmul(
                            psF[:, gi * hw:(gi + 1) * hw],
                            lhsT=swf[:, 152 + (si * 4 + gg) * 128:
                                     152 + (si * 4 + gg + 1) * 128],
                            rhs=Hn[:, 0:hw], start=True, stop=True)
                    cols = (4 // ntl) * hw
                    if si == 0 and tl == 0:
                        nc.vector.tensor_copy(out=fm[:, 0:cols],
                                              in_=psF[:, 0:cols])
                    else:
                        nc.scalar.copy(out=fm[:, tl * cols:(tl + 1) * cols],
                                       in_=psF[:, 0:cols])
                fstate[100 + si] = fm

            def hb_copy():
                Hb = sb.tile([128, 336], bf16, tag="Hb")
                nc.vector.tensor_copy(out=Hb, in_=heat[:, HOFF[3]:GB_TOT])
                fstate["Hb"] = Hb

            events = {
                1: lambda: hb_copy(),
                2: lambda: (fm_sel(0),
                            nc.gpsimd.dma_start(out=oreg(3),
                                                in_=heat[:, HOFF[3]:GB_TOT])),
                3: lambda: (fm_rep(0),
                            nc.gpsimd.dma_start(out=oreg(2),
                                                in_=heat[:, HOFF[2]:HOFF[3]])),
                4: lambda: fm_mm(0),
                5: lambda: fm_sel(1),
                6: lambda: fm_rep(1),
                7: lambda: fm_mm(1),
                8: lambda: fm_sel(2),
                9: lambda: fm_rep(2),
                10: lambda: fm_mm(2),
                11: lambda: (
                    nc.gpsimd.dma_start(out=oreg(4), in_=fstate[100]),
                    nc.gpsimd.dma_start(out=oreg(5), in_=fstate[101]),
                    nc.gpsimd.dma_start(out=oreg(1),
                                        in_=heat[:, HOFF[1]:HOFF[2]])),
                12: lambda: nc.gpsimd.dma_start(out=oreg(6), in_=fstate[102]),
            }

            ntile = len(GEN_TILES)
            for ti, (off, n, band) in enumerate(GEN_TILES):
                ps = pgen.tile([128, 512], f32, tag="ps")
                lhsT = czzA if band == 0 else czzB
                gc = off - BSPLIT if band == 0 else off
                nc.tensor.matmul(ps[:, 0:n], lhsT=lhsT,
                                 rhs=sgb[:, gc:gc + n],
                                 start=True, stop=True)
                if ti == ntile - 1:
                    recip_dve(off, n, ps[:, 0:n])
                elif ti % 2 == 0:
                    recip_dve(off, n, ps[:, 0:n])
                else:
                    recip_act(off, n, ps[:, 0:n])
                if ti in events:
                    events[ti]()

            # stage-0 part heat: everything below heat col 16384 done last
            nc.sync.dma_start(out=oreg(0), in_=heat[:, HOFF[0]:HOFF[1]])
    nc.compile()
    return nc


def _get_nc():
    if "nc" not in _NC_CACHE:
        _NC_CACHE["nc"] = _build()
    return _NC_CACHE["nc"]


def _in_maps(part_maps, features):
    part_maps = np.asarray(part_maps, dtype=np.float32)
    features = np.asarray(features, dtype=np.float32)
    gb12, mb, cst, selb, repb = _host_consts()
    in_maps = []
    for core in range(NCORES):
        pm = part_maps[core * BL:(core + 1) * BL]
        pmr = pm.transpose(1, 0, 2, 3).reshape(ROWS, HMAP * HMAP)
        ptm = pmr.reshape(128, 32, 128).transpose(2, 1, 0).reshape(128, 4096)
        pta = np.concatenate([mb, ptm], axis=1).astype(np.float32)
        pta = np.ascontiguousarray(pta)
        wf = _host_wf(features[core * BL:(core + 1) * BL])
        wfp = np.concatenate([selb, repb, wf], axis=1)
        in_maps.append({"pta": pta, "gbb": gb12, "cstb": cst, "wfb": wfp})
    return in_maps


def _unpack(arr):
    """arr: [OUT_ROWS, STP] padded f32 -> (BL, OUT_TOT) for one core."""
    out = np.empty((BL, OUT_TOT), dtype=np.float32)

    def reg(ri, shape):
        r0, r1 = int(REGION_ROFF[ri]), int(REGION_ROFF[ri + 1])
        return arr[r0:r1, 0:REGION_W[ri]].reshape(shape)

    for s in (0, 1, 2):
        hw = HWS[s]
        block = reg(s, (NK, BL, hw))
        out[:, OUT_PH[s]:OUT_PH[s] + NK * hw] = \
            block.transpose(1, 0, 2).reshape(BL, NK * hw)
    b345 = reg(3, (128, 336))
    for s, c0 in ((3, 0), (4, 256), (5, 320)):
        pd = STAGES[s][2]
        hw = HWS[s]
        blk = b345[0:pd * BL, c0:c0 + hw].reshape(pd, BL, hw)
        out[:, OUT_PH[s]:OUT_PH[s] + pd * hw] = \
            blk.transpose(1, 0, 2).reshape(BL, pd * hw)
    for si, sidx in enumerate((3, 4, 5)):
        hw = HWS[sidx]
        block = reg(4 + si, (128, 4 * hw))
        fb = block.reshape(2, NF, 4, hw)             # (bo, n, g, f)
        fmap = fb.transpose(2, 0, 1, 3).reshape(BL, NF * hw)
        out[:, OUT_FM[sidx]:OUT_FM[sidx] + NF * hw] = fmap
    return out


def _run(part_maps, features, trace=False):
    from concourse.bass_utils import run_bass_kernel_spmd
    nc = _get_nc()
    res = run_bass_kernel_spmd(nc, _in_maps(part_maps, features),
                               list(range(NCORES)), trace=trace)
    outs = [_unpack(res.results[i]["out"]) for i in range(NCORES)]
    return np.concatenate(outs, axis=0), res


def kernel(part_maps, features):
    out, _ = _run(part_maps, features, trace=False)
    return out


# revision 9
# speedup vs baseline: 4.6505x; 1.0401x over previous
"""Trainium2 Bass kernel v4 for the part-map heatmap-pyramid encoder.

Contract: kernel(part_maps, features) -> (64, 369952) float32.
Data parallel over batch: 8 samples per NeuronCore x 8 cores.

Per-core pipeline:
  1. moments: 32 accumulating fp32 matmuls over pixel chunks -> [128, 5];
     moment basis is folded into the same DRAM load as the part maps.
  2. coefficient chain split over Pool+DVE (no sqrt: L_inv^T L_inv =
     (0.8/det)^2 [[s11,-s01],[-s01,s00]]), one divide.
  3. transpose coef -> [6, 128] bf16.
  4. gen: rank-6 bf16 matmuls (PE) from a resident [6, 21840] bf16 basis
     into [128, 1024] PSUM tiles; heat = 1/proj via Pool divide / DVE
     reciprocal_approx_fast; all heat in one [128, 21840] f32 SBUF tile.
  5. fmap chains for stages 3-5, interleaved into the gen stream.
  6. stores: padded flat DRAM regions (500ns each in-model), host unpack.
"""

import numpy as np

BN, NK, NF, HMAP = 64, 16, 64, 64
NCORES = 8
BL = BN // NCORES            # samples per core = 8
ROWS = BL * NK               # partition rows per core = 128
L_INV_SCAL = 0.8
EPS_DIST = 1e-6
EPS_COV = 1e-12

STAGES = [(128, 128, NK, (0, 0)), (64, 64, NK, (0, 0)), (32, 32, NK, (0, 0)),
          (16, 16, NK, (4, NK)), (8, 8, 4, (2, 4)), (4, 4, 2, (0, 2))]
HWS = [h * w for (h, w, _, _) in STAGES]
HOFF = np.concatenate([[0], np.cumsum(HWS)]).astype(int)
GB_TOT = int(HOFF[-1])       # 21840

_off = 0
OUT_PH = []
OUT_FM = []
for (h, w, pd, (s0, s1)) in STAGES:
    OUT_PH.append(_off)
    _off += pd * h * w
    if s1 - s0 != 0:
        OUT_FM.append(_off)
        _off += NF * h * w
    else:
        OUT_FM.append(None)
OUT_TOT = _off               # 369952

# ---- padded output regions: [elems/64 rows, 80] each (64 data cols) ----
STP = 80
# R0, R1, R2 (stage 0-2 part heats), R345 (stages 3-5 cols incl garbage
# rows), F3, F4, F5 (fmap tiles)
REGION_ELEMS = [128 * HWS[0], 128 * HWS[1], 128 * HWS[2], 128 * 336,
                128 * 4 * HWS[3], 128 * 4 * HWS[4], 128 * 4 * HWS[5]]
REGION_W = [64, 64, 64, 16, 64, 64, 64]
REGION_ROWS = [e // w for e, w in zip(REGION_ELEMS, REGION_W)]
REGION_ROFF = np.concatenate([[0], np.cumsum(REGION_ROWS)]).astype(int)
OUT_ROWS = int(REGION_ROFF[-1])

# 2-band basis split point (heat cols >= BSPLIT live in sgb12 rows 0-5)
BSPLIT = 10920
# gen tiles: (heat_offset, ncols, band): band A ordered stage345 tail,
# stage2, stage1, stage0-back; then band B = stage0 front
GEN_TILES = [(int(HOFF[3]), 336, 0)]
for c in range(int(HOFF[2]), int(HOFF[3]), 512):
    GEN_TILES.append((c, 512, 0))
for c in range(int(HOFF[1]), int(HOFF[2]), 512):
    GEN_TILES.append((c, 512, 0))
for c in range(BSPLIT, int(HOFF[1]), 512):
    GEN_TILES.append((c, min(512, int(HOFF[1]) - c), 0))
for c in range(0, BSPLIT, 512):
    GEN_TILES.append((c, min(512, BSPLIT - c), 1))


def _mesh_basis(h, w):
    y = np.linspace(-1.0, 1.0, h, dtype=np.float64)
    x = np.linspace(-1.0, 1.0, w, dtype=np.float64)
    yy = np.repeat(y, w)
    xx = np.tile(x, h)
    return np.stack([np.ones_like(yy), yy, xx, yy * yy, yy * xx, xx * xx])


def _bf16(a):
    import ml_dtypes
    return np.asarray(a, dtype=np.float32).astype(ml_dtypes.bfloat16)


def _host_consts():
    gb = np.concatenate([_mesh_basis(h, w) for (h, w, _, _) in STAGES], axis=1)
    # 2-band layout [12, BSPLIT]: rows 0-5 = cols BSPLIT.., rows 6-11 = front
    gb12 = np.concatenate([gb[:, BSPLIT:], gb[:, 0:BSPLIT]], axis=0)
    bm = _mesh_basis(HMAP, HMAP)[1:6]              # [5, 4096]
    mb = np.zeros((128, 160), dtype=np.float32)
    for c in range(32):
        mb[:, c * 5:(c + 1) * 5] = bm[:, c * 128:(c + 1) * 128].T
    cst = np.eye(128, dtype=np.float32)
    sel = np.zeros((128, 24), dtype=np.float32)
    rep = np.zeros((128, 128), dtype=np.float32)
    for b in range(BL):
        for k in range(NK):
            rep[b, k * 8 + b] = 1.0
        for si, sidx in enumerate((3, 4, 5)):
            s0, s1 = STAGES[sidx][3]
            for k in range(s0, s1):
                sel[k * 8 + b, si * 8 + b] = 1.0
    return _bf16(gb12), mb, cst, _bf16(sel), _bf16(rep)


def _host_wf(features_core):
    wf = np.zeros((128, 12 * 128), dtype=np.float32)
    for si, sidx in enumerate((3, 4, 5)):
        s0, s1 = STAGES[sidx][3]
        for g in range(4):
            blk = (si * 4 + g) * 128
            for bo in range(2):
                b = 2 * g + bo
                for k in range(s0, s1):
                    wf[k * 8 + b, blk + 64 * bo:blk + 64 * (bo + 1)] = \
                        features_core[b, k, :]
    return _bf16(wf)


_NC_CACHE = {}


def _build():
    import concourse.bass as bass
    import concourse.bacc as bacc
    import concourse.tile as tile
    from concourse import mybir

    f32 = mybir.dt.float32
    bf16 = mybir.dt.bfloat16
    AT = mybir.AluOpType

    nc = bacc.Bacc("TRN2", target_bir_lowering=False, debug=False)
    # pta: [mb(160) | pt(4096)] f32
    pta = nc.declare_dram_parameter("pta", [128, 4256], f32, isOutput=False)
    gbb = nc.declare_dram_parameter("gbb", [12, BSPLIT], bf16, isOutput=False)
    cstb = nc.declare_dram_parameter("cstb", [128, 128], f32, isOutput=False)
    wfb = nc.declare_dram_parameter("wfb", [128, 1688], bf16, isOutput=False)
    outp = nc.declare_dram_parameter("out", [OUT_ROWS, STP], f32,
                                     isOutput=True)

    def oreg(ri):
        r0, r1 = int(REGION_ROFF[ri]), int(REGION_ROFF[ri + 1])
        return outp[r0:r1, 0:REGION_W[ri]]

    with tile.TileContext(nc) as tc:
        import contextlib
        ctx = contextlib.ExitStack()
        with ctx:
            sb = ctx.enter_context(tc.tile_pool(name="sb", bufs=1))
            pgen = ctx.enter_context(tc.tile_pool(name="pgen", bufs=6,
                                                  space="PSUM"))
            pmisc = ctx.enter_context(tc.tile_pool(name="pmisc", bufs=1,
                                                   space="PSUM"))

            spta = sb.tile([128, 4256], f32, tag="spta")
            scst = sb.tile([128, 128], f32, tag="scst")
            sgb = sb.tile([12, BSPLIT], bf16, tag="sgb")
            swf = sb.tile([128, 1688], bf16, tag="swf")

            # pt pieces: [0:1184]=mb+chunks0-7, then 1024-col pieces
            nc.sync.dma_start(out=spta[:, 0:1184], in_=pta[:, 0:1184])
            nc.scalar.dma_start(out=spta[:, 1184:2208], in_=pta[:, 1184:2208])
            nc.gpsimd.dma_start(out=spta[:, 2208:3232], in_=pta[:, 2208:3232])
            nc.gpsimd.dma_start(out=spta[:, 3232:4256], in_=pta[:, 3232:4256])
            # basis on SP in gen-consumption order (high cols first);
            # Act only gets small early loads so its recip window is clear
            nc.scalar.dma_start(out=scst, in_=cstb[:, :])
            nc.sync.dma_start(out=sgb[:, 9560:BSPLIT],
                              in_=gbb[:, 9560:BSPLIT])
            nc.scalar.dma_start(out=swf[:, 0:664], in_=wfb[:, 0:664])
            nc.scalar.dma_start(out=swf[:, 664:1688], in_=wfb[:, 664:1688])
            nc.sync.dma_start(out=sgb[:, 5460:7508], in_=gbb[:, 5460:7508])
            nc.sync.dma_start(out=sgb[:, 7508:9560], in_=gbb[:, 7508:9560])
            nc.gpsimd.dma_start(out=sgb[:, 0:2730], in_=gbb[:, 0:2730])
            nc.gpsimd.dma_start(out=sgb[:, 2730:5460], in_=gbb[:, 2730:5460])

            czzA = sb.tile([12, 128], bf16, tag="czzA")
            nc.gpsimd.memset(czzA, 0.0)
            czzB = sb.tile([12, 128], bf16, tag="czzB")
            nc.gpsimd.memset(czzB, 0.0)

            jnk = sb.tile([1, 4], f32, tag="jnk")
            nc.scalar.copy(out=jnk, in_=scst[0:1, 0:4])

            # ---- moments (with PE pstate-warming fillers) ----
            def warm(k):
                for _ in range(k):
                    pw = pgen.tile([128, 64], f32, tag="ps", name="pw")
                    nc.tensor.matmul(pw, lhsT=spta[:, 160:288],
                                     rhs=spta[:, 0:64], start=True, stop=True)

            psmom = pmisc.tile([128, 8], f32, tag="m1")
            for n_, c in enumerate(range(32)):
                nc.tensor.matmul(
                    psmom[:, 0:5],
                    lhsT=spta[:, 160 + c * 128:160 + (c + 1) * 128],
                    rhs=spta[:, c * 5:(c + 1) * 5],
                    start=(n_ == 0),
                    stop=(n_ == 31),
                )
                if c in (7, 15, 23):
                    warm(2)
            warm(8)

            # ---- coefficient chain, Pool (g) / DVE (v) split ----
            def t(cols, tag):
                return sb.tile([128, cols], f32, tag=tag, name=tag)

            v = nc.vector
            g = v
            mom = t(5, "mom")
            v.tensor_copy(out=mom, in_=psmom[:, 0:5])  # muy mux Eyy Eyx Exx
            pyy = t(1, "pyy"); pyx_ = t(1, "pyx_"); pxx = t(1, "pxx")
            g.tensor_tensor(out=pyy, in0=mom[:, 0:1], in1=mom[:, 0:1],
                            op=AT.mult)
            g.tensor_tensor(out=pyx_, in0=mom[:, 0:1], in1=mom[:, 1:2],
                            op=AT.mult)
            g.tensor_tensor(out=pxx, in0=mom[:, 1:2], in1=mom[:, 1:2],
                            op=AT.mult)
            c00 = t(1, "c00"); c01 = t(1, "c01"); c11 = t(1, "c11")
            g.tensor_tensor(out=c00, in0=mom[:, 2:3], in1=pyy, op=AT.subtract)
            g.tensor_tensor(out=c01, in0=mom[:, 3:4], in1=pyx_,
                            op=AT.subtract)
            g.tensor_tensor(out=c11, in0=mom[:, 4:5], in1=pxx, op=AT.subtract)
            pp = t(2, "pp")
            v.tensor_scalar(out=pp, in0=mom[:, 0:2], scalar1=-1.0,
                            scalar2=EPS_DIST, op0=AT.mult, op1=AT.add)
            qyy = t(1, "qyy"); qyx = t(1, "qyx"); qxx = t(1, "qxx")
            v.tensor_tensor(out=qyy, in0=pp[:, 0:1], in1=pp[:, 0:1],
                            op=AT.mult)
            v.tensor_tensor(out=qyx, in0=pp[:, 0:1], in1=pp[:, 1:2],
                            op=AT.mult)
            v.tensor_tensor(out=qxx, in0=pp[:, 1:2], in1=pp[:, 1:2],
                            op=AT.mult)
            m1_ = t(1, "m1_"); m2_ = t(1, "m2_")
            g.tensor_tensor(out=m1_, in0=c00, in1=c11, op=AT.mult)
            g.tensor_tensor(out=m2_, in0=c01, in1=c01, op=AT.mult)
            d2 = t(1, "d2")
            g.tensor_tensor(out=d2, in0=m1_, in1=m2_, op=AT.subtract)
            d2s = t(1, "d2s")
            g.tensor_scalar(out=d2s, in0=d2,
                            scalar1=1.0 / (L_INV_SCAL * L_INV_SCAL),
                            scalar2=1e-20, op0=AT.mult, op1=AT.add)
            q = t(1, "q")
            v.reciprocal_approx_fast(out=q, in_=d2s)

            coef = sb.tile([128, 6], f32, tag="coef")
            g.tensor_tensor(out=coef[:, 3:4], in0=q, in1=c11, op=AT.mult)
            g.scalar_tensor_tensor(out=coef[:, 4:5], in0=q, scalar=-2.0,
                                   in1=c01, op0=AT.mult, op1=AT.mult)
            g.tensor_tensor(out=coef[:, 5:6], in0=q, in1=c00, op=AT.mult)
            tA = t(1, "tA"); tB = t(1, "tB"); tC = t(1, "tC")
            v.tensor_tensor(out=tA, in0=coef[:, 3:4], in1=qyy, op=AT.mult)
            v.tensor_tensor(out=tB, in0=coef[:, 4:5], in1=qyx, op=AT.mult)
            v.tensor_tensor(out=tC, in0=coef[:, 5:6], in1=qxx, op=AT.mult)
            c0a = t(1, "c0a")
            v.tensor_tensor(out=c0a, in0=tA, in1=tB, op=AT.add)
            c0b = t(1, "c0b")
            v.tensor_tensor(out=c0b, in0=c0a, in1=tC, op=AT.add)
            v.tensor_scalar_add(out=coef[:, 0:1], in0=c0b, scalar1=1.0)
            t4 = t(1, "t4"); t5 = t(1, "t5")
            g.tensor_tensor(out=t4, in0=coef[:, 3:4], in1=pp[:, 0:1],
                            op=AT.mult)
            g.tensor_tensor(out=t5, in0=coef[:, 4:5], in1=pp[:, 1:2],
                            op=AT.mult)
            g.scalar_tensor_tensor(out=coef[:, 1:2], in0=t4, scalar=2.0,
                                   in1=t5, op0=AT.mult, op1=AT.add)
            t6 = t(1, "t6"); t7 = t(1, "t7")
            g.tensor_tensor(out=t6, in0=coef[:, 4:5], in1=pp[:, 0:1],
                            op=AT.mult)
            g.tensor_tensor(out=t7, in0=coef[:, 5:6], in1=pp[:, 1:2],
                            op=AT.mult)
            g.scalar_tensor_tensor(out=coef[:, 2:3], in0=t7, scalar=2.0,
                                   in1=t6, op0=AT.mult, op1=AT.add)

            pst = pmisc.tile([6, 128], f32, tag="m1")
            nc.tensor.transpose(pst, coef, scst[:, 0:128])
            coefT = sb.tile([6, 128], bf16, tag="coefT")
            nc.vector.tensor_copy(out=coefT, in_=pst)
            # band lhsT variants: A = [coefT; 0] (legal partition-32 copy),
            # B = [0; coefT] (SBUF->SBUF DMA; latency hides under band A)
            nc.vector.tensor_copy(out=czzA[0:6, :], in_=coefT)
            nc.sync.dma_start(out=czzB[6:12, :], in_=coefT)

            # ---- heat generation ----
            heat = sb.tile([128, GB_TOT], f32, tag="heat")

            def recip_dve(off, n, ps):
                nc.vector.reciprocal_approx_fast(
                    out=heat[:, off:off + n], in_=ps)

            def recip_act(off, n, ps):
                se = nc.scalar
                se.add_instruction(
                    mybir.InstActivation(
                        name=nc.get_next_instruction_name(),
                        func=mybir.ActivationFunctionType.Reciprocal,
                        ins=[se.lower_ap(ps),
                             mybir.ImmediateValue(dtype=f32, value=0.0),
                             mybir.ImmediateValue(dtype=f32, value=1.0),
                             mybir.ImmediateValue(dtype=f32, value=0.0)],
                        outs=[se.lower_ap(heat[:, off:off + n])],
                    ))

            # ---- fmap chains (bf16), fed from an Hb bf16 copy ----
            fstate = {}
            HB0 = int(HOFF[3])

            def fm_sel(si):
                hw = HWS[3 + si]
                hb0 = int(HOFF[3 + si]) - HB0
                pss = pmisc.tile([8, 256], f32, tag="m1")
                nc.tensor.matmul(pss[:, 0:hw],
                                 lhsT=swf[:, si * 8:si * 8 + 8],
                                 rhs=fstate["Hb"][:, hb0:hb0 + hw],
                                 start=True, stop=True)
                rt = sb.tile([8, 256], f32, tag="rt", bufs=2)
                nc.vector.tensor_scalar_add(out=rt[:, 0:hw], in0=pss[:, 0:hw],
                                            scalar1=1.0)
                rr = sb.tile([8, 256], f32, tag="rr", bufs=2)
                nc.vector.reciprocal_approx_fast(out=rr[:, 0:hw],
                                                 in_=rt[:, 0:hw])
                rrb = sb.tile([8, 256], bf16, tag="rrb", bufs=2)
                nc.vector.tensor_copy(out=rrb[:, 0:hw], in_=rr[:, 0:hw])
                fstate[si] = (rrb, hw)

            def fm_rep(si):
                rrb, hw = fstate[si]
                H = heat[:, int(HOFF[3 + si]):int(HOFF[3 + si]) + hw]
                psR = pmisc.tile([128, 256], f32, tag="m1")
                nc.tensor.matmul(psR[:, 0:hw], lhsT=swf[0:8, 24:152],
                                 rhs=rrb[:, 0:hw], start=True, stop=True)
                Hn = sb.tile([128, 256], bf16, tag="Hn", bufs=2)
                nc.vector.tensor_tensor(out=Hn[:, 0:hw], in0=H,
                                        in1=psR[:, 0:hw], op=AT.mult)
                fstate[si] = (Hn, hw)

            def fm_mm(si):
                Hn, hw = fstate[si]
                fm = sb.tile([128, 4 * hw], f32, tag=f"fm{si}")
                ntl = 2 if si == 0 else 1
                for tl in range(ntl):
                    psF = pmisc.tile([128, 512], f32, tag="psF")
                    for gi in range(4 // ntl):
                        gg = tl * (4 // ntl) + gi
                        nc.tensor.matmul(
                            psF[:, gi * hw:(gi + 1) * hw],
                            lhsT=swf[:, 152 + (si * 4 + gg) * 128:
                                     152 + (si * 4 + gg + 1) * 128],
                            rhs=Hn[:, 0:hw], start=True, stop=True)
                    cols = (4 // ntl) * hw
                    if si == 0 and tl == 0:
                        nc.vector.tensor_copy(out=fm[:, 0:cols],
                                              in_=psF[:, 0:cols])
                    else:
                        nc.scalar.copy(out=fm[:, tl * cols:(tl + 1) * cols],
                                       in_=psF[:, 0:cols])
                fstate[100 + si] = fm

            def hb_copy():
                Hb = sb.tile([128, 336], bf16, tag="Hb")
                nc.vector.tensor_copy(out=Hb, in_=heat[:, HOFF[3]:GB_TOT])
                fstate["Hb"] = Hb

            events = {
                1: lambda: hb_copy(),
                2: lambda: (fm_sel(0),
                            nc.gpsimd.dma_start(out=oreg(3),
                                                in_=heat[:, HOFF[3]:GB_TOT])),
                3: lambda: (fm_rep(0),
                            nc.gpsimd.dma_start(out=oreg(2),
                                                in_=heat[:, HOFF[2]:HOFF[3]])),
                4: lambda: fm_mm(0),
                5: lambda: fm_sel(1),
                6: lambda: fm_rep(1),
                7: lambda: fm_mm(1),
                8: lambda: fm_sel(2),
                9: lambda: fm_rep(2),
                10: lambda: fm_mm(2),
                11: lambda: (
                    nc.gpsimd.dma_start(out=oreg(4), in_=fstate[100]),
                    nc.gpsimd.dma_start(out=oreg(5), in_=fstate[101]),
                    nc.gpsimd.dma_start(out=oreg(1),
                                        in_=heat[:, HOFF[1]:HOFF[2]])),
                12: lambda: nc.gpsimd.dma_start(out=oreg(6), in_=fstate[102]),
            }

            ntile = len(GEN_TILES)
            for ti, (off, n, band) in enumerate(GEN_TILES):
                ps = pgen.tile([128, 512], f32, tag="ps")
                lhsT = czzA if band == 0 else czzB
                gc = off - BSPLIT if band == 0 else off
                nc.tensor.matmul(ps[:, 0:n], lhsT=lhsT,
                                 rhs=sgb[:, gc:gc + n],
                                 start=True, stop=True)
                if ti == ntile - 1:
                    recip_dve(off, n, ps[:, 0:n])
                else:
                    PAT11 = (recip_dve, recip_act, recip_dve, recip_act,
                             recip_dve, recip_act, recip_act, recip_dve,
                             recip_act, recip_dve, recip_act)
                    PAT11[ti % 11](off, n, ps[:, 0:n])
                if ti in events:
                    events[ti]()

            # stage-0 part heat: everything below heat col 16384 done last
            nc.sync.dma_start(out=oreg(0), in_=heat[:, HOFF[0]:HOFF[1]])
    nc.compile()
    return nc


def _get_nc():
    if "nc" not in _NC_CACHE:
        _NC_CACHE["nc"] = _build()
    return _NC_CACHE["nc"]


def _in_maps(part_maps, features):
    part_maps = np.asarray(part_maps, dtype=np.float32)
    features = np.asarray(features, dtype=np.float32)
    gb12, mb, cst, selb, repb = _host_consts()
    in_maps = []
    for core in range(NCORES):
        pm = part_maps[core * BL:(core + 1) * BL]
        pmr = pm.transpose(1, 0, 2, 3).reshape(ROWS, HMAP * HMAP)
        ptm = pmr.reshape(128, 32, 128).transpose(2, 1, 0).reshape(128, 4096)
        pta = np.concatenate([mb, ptm], axis=1).astype(np.float32)
        pta = np.ascontiguousarray(pta)
        wf = _host_wf(features[core * BL:(core + 1) * BL])
        wfp = np.concatenate([selb, repb, wf], axis=1)
        in_maps.append({"pta": pta, "gbb": gb12, "cstb": cst, "wfb": wfp})
    return in_maps


def _unpack(arr):
    """arr: [OUT_ROWS, STP] padded f32 -> (BL, OUT_TOT) for one core."""
    out = np.empty((BL, OUT_TOT), dtype=np.float32)

    def reg(ri, shape):
        r0, r1 = int(REGION_ROFF[ri]), int(REGION_ROFF[ri + 1])
        return arr[r0:r1, 0:REGION_W[ri]].reshape(shape)

    for s in (0, 1, 2):
        hw = HWS[s]
        block = reg(s, (NK, BL, hw))
        out[:, OUT_PH[s]:OUT_PH[s] + NK * hw] = \
            block.transpose(1, 0, 2).reshape(BL, NK * hw)
    b345 = reg(3, (128, 336))
    for s, c0 in ((3, 0), (4, 256), (5, 320)):
        pd = STAGES[s][2]
        hw = HWS[s]
        blk = b345[0:pd * BL, c0:c0 + hw].reshape(pd, BL, hw)
        out[:, OUT_PH[s]:OUT_PH[s] + pd * hw] = \
            blk.transpose(1, 0, 2).reshape(BL, pd * hw)
    for si, sidx in enumerate((3, 4, 5)):
        hw = HWS[sidx]
        block = reg(4 + si, (128, 4 * hw))
        fb = block.reshape(2, NF, 4, hw)             # (bo, n, g, f)
        fmap = fb.transpose(2, 0, 1, 3).reshape(BL, NF * hw)
        out[:, OUT_FM[sidx]:OUT_FM[sidx] + NF * hw] = fmap
    return out


def _run(part_maps, features, trace=False):
    from concourse.bass_utils import run_bass_kernel_spmd
    nc = _get_nc()
    res = run_bass_kernel_spmd(nc, _in_maps(part_maps, features),
                               list(range(NCORES)), trace=trace)
    outs = [_unpack(res.results[i]["out"]) for i in range(NCORES)]
    return np.concatenate(outs, axis=0), res


def kernel(part_maps, features):
    out, _ = _run(part_maps, features, trace=False)
    return out


# revision 16
# speedup vs baseline: 5.0368x; 1.0831x over previous
"""Trainium2 Bass kernel v4 for the part-map heatmap-pyramid encoder.

Contract: kernel(part_maps, features) -> (64, 369952) float32.
Data parallel over batch: 8 samples per NeuronCore x 8 cores.

Per-core pipeline:
  1. moments: 32 accumulating fp32 matmuls over pixel chunks -> [128, 5];
     moment basis is folded into the same DRAM load as the part maps.
  2. coefficient chain split over Pool+DVE (no sqrt: L_inv^T L_inv =
     (0.8/det)^2 [[s11,-s01],[-s01,s00]]), one divide.
  3. transpose coef -> [6, 128] bf16.
  4. gen: rank-6 bf16 matmuls (PE) from a resident [6, 21840] bf16 basis
     into [128, 1024] PSUM tiles; heat = 1/proj via Pool divide / DVE
     reciprocal_approx_fast; all heat in one [128, 21840] f32 SBUF tile.
  5. fmap chains for stages 3-5, interleaved into the gen stream.
  6. stores: padded flat DRAM regions (500ns each in-model), host unpack.
"""

import numpy as np

BN, NK, NF, HMAP = 64, 16, 64, 64
NCORES = 8
BL = BN // NCORES            # samples per core = 8
ROWS = BL * NK               # partition rows per core = 128
L_INV_SCAL = 0.8
EPS_DIST = 1e-6
EPS_COV = 1e-12

STAGES = [(128, 128, NK, (0, 0)), (64, 64, NK, (0, 0)), (32, 32, NK, (0, 0)),
          (16, 16, NK, (4, NK)), (8, 8, 4, (2, 4)), (4, 4, 2, (0, 2))]
HWS = [h * w for (h, w, _, _) in STAGES]
HOFF = np.concatenate([[0], np.cumsum(HWS)]).astype(int)
GB_TOT = int(HOFF[-1])       # 21840

_off = 0
OUT_PH = []
OUT_FM = []
for (h, w, pd, (s0, s1)) in STAGES:
    OUT_PH.append(_off)
    _off += pd * h * w
    if s1 - s0 != 0:
        OUT_FM.append(_off)
        _off += NF * h * w
    else:
        OUT_FM.append(None)
OUT_TOT = _off               # 369952

# ---- padded output regions: [elems/64 rows, 80] each (64 data cols) ----
STP = 80
# R0, R1, R2 (stage 0-2 part heats), R345 (stages 3-5 cols incl garbage
# rows), F3, F4, F5 (fmap tiles)
REGION_ELEMS = [128 * HWS[0], 128 * HWS[1], 128 * HWS[2], 128 * 336,
                128 * 4 * HWS[3], 128 * 4 * HWS[4], 128 * 4 * HWS[5]]
REGION_W = [64, 64, 64, 16, 64, 64, 64]
REGION_ROWS = [e // w for e, w in zip(REGION_ELEMS, REGION_W)]
REGION_ROFF = np.concatenate([[0], np.cumsum(REGION_ROWS)]).astype(int)
OUT_ROWS = int(REGION_ROFF[-1])

# 2-band basis split point (heat cols >= BSPLIT live in sgb12 rows 0-5)
BSPLIT = 10920
# gen tiles: (heat_offset, ncols, band): band A ordered stage345 tail,
# stage2, stage1, stage0-back; then band B = stage0 front
GEN_TILES = [(int(HOFF[3]), 336, 0)]
for c in range(int(HOFF[2]), int(HOFF[3]), 512):
    GEN_TILES.append((c, 512, 0))
for c in range(int(HOFF[1]), int(HOFF[2]), 512):
    GEN_TILES.append((c, 512, 0))
for c in range(BSPLIT, int(HOFF[1]), 512):
    GEN_TILES.append((c, min(512, int(HOFF[1]) - c), 0))
for c in range(0, BSPLIT, 512):
    GEN_TILES.append((c, min(512, BSPLIT - c), 1))


def _mesh_basis(h, w):
    y = np.linspace(-1.0, 1.0, h, dtype=np.float64)
    x = np.linspace(-1.0, 1.0, w, dtype=np.float64)
    yy = np.repeat(y, w)
    xx = np.tile(x, h)
    return np.stack([np.ones_like(yy), yy, xx, yy * yy, yy * xx, xx * xx])


def _bf16(a):
    import ml_dtypes
    return np.asarray(a, dtype=np.float32).astype(ml_dtypes.bfloat16)


def _host_consts():
    gb = np.concatenate([_mesh_basis(h, w) for (h, w, _, _) in STAGES], axis=1)
    # 2-band layout [12, BSPLIT]: rows 0-5 = cols BSPLIT.., rows 6-11 = front
    gb12 = np.concatenate([gb[:, BSPLIT:], gb[:, 0:BSPLIT]], axis=0)
    bm = _mesh_basis(HMAP, HMAP)[1:6]              # [5, 4096]
    mb = np.zeros((128, 160), dtype=np.float32)
    for c in range(32):
        mb[:, c * 5:(c + 1) * 5] = bm[:, c * 128:(c + 1) * 128].T
    cst = np.eye(128, dtype=np.float32)
    sel = np.zeros((128, 24), dtype=np.float32)
    rep = np.zeros((128, 128), dtype=np.float32)
    for b in range(BL):
        for k in range(NK):
            rep[b, k * 8 + b] = 1.0
        for si, sidx in enumerate((3, 4, 5)):
            s0, s1 = STAGES[sidx][3]
            for k in range(s0, s1):
                sel[k * 8 + b, si * 8 + b] = 1.0
    return _bf16(gb12), mb, cst, _bf16(sel), _bf16(rep)


def _host_wf(features_core):
    wf = np.zeros((128, 12 * 128), dtype=np.float32)
    for si, sidx in enumerate((3, 4, 5)):
        s0, s1 = STAGES[sidx][3]
        for g in range(4):
            blk = (si * 4 + g) * 128
            for bo in range(2):
                b = 2 * g + bo
                for k in range(s0, s1):
                    wf[k * 8 + b, blk + 64 * bo:blk + 64 * (bo + 1)] = \
                        features_core[b, k, :]
    return _bf16(wf)


_NC_CACHE = {}


def _build():
    import concourse.bass as bass
    import concourse.bacc as bacc
    import concourse.tile as tile
    from concourse import mybir

    f32 = mybir.dt.float32
    bf16 = mybir.dt.bfloat16
    AT = mybir.AluOpType

    nc = bacc.Bacc("TRN2", target_bir_lowering=False, debug=False)
    # pta: [mb(160) | pt(4096)] f32
    pta = nc.declare_dram_parameter("pta", [128, 4256], f32, isOutput=False)
    gbb = nc.declare_dram_parameter("gbb", [12, BSPLIT], bf16, isOutput=False)
    cstb = nc.declare_dram_parameter("cstb", [128, 128], f32, isOutput=False)
    wfb = nc.declare_dram_parameter("wfb", [128, 1688], bf16, isOutput=False)
    outp = nc.declare_dram_parameter("out", [OUT_ROWS, STP], f32,
                                     isOutput=True)

    def oreg(ri):
        r0, r1 = int(REGION_ROFF[ri]), int(REGION_ROFF[ri + 1])
        return outp[r0:r1, 0:REGION_W[ri]]

    with tile.TileContext(nc) as tc:
        import contextlib
        ctx = contextlib.ExitStack()
        with ctx:
            sb = ctx.enter_context(tc.tile_pool(name="sb", bufs=1))
            pgen = ctx.enter_context(tc.tile_pool(name="pgen", bufs=6,
                                                  space="PSUM"))
            pmisc = ctx.enter_context(tc.tile_pool(name="pmisc", bufs=1,
                                                   space="PSUM"))

            czzA = sb.tile([12, 128], bf16, tag="czzA")
            nc.gpsimd.memset(czzA, 0.0)
            czzB = sb.tile([12, 128], bf16, tag="czzB")
            nc.gpsimd.memset(czzB, 0.0)
            spta = sb.tile([128, 4256], f32, tag="spta")
            scst = sb.tile([128, 128], f32, tag="scst")
            sgb = sb.tile([12, BSPLIT], bf16, tag="sgb")
            swf = sb.tile([128, 1688], bf16, tag="swf")

            # pt pieces: [0:1184]=mb+chunks0-7, then 1024-col pieces
            nc.sync.dma_start(out=spta[:, 0:1184], in_=pta[:, 0:1184])
            nc.scalar.dma_start(out=spta[:, 1184:2208], in_=pta[:, 1184:2208])
            nc.gpsimd.dma_start(out=spta[:, 2208:3232], in_=pta[:, 2208:3232])
            nc.gpsimd.dma_start(out=spta[:, 3232:4256], in_=pta[:, 3232:4256])
            # basis on SP in gen-consumption order (high cols first);
            # Act only gets small early loads so its recip window is clear
            nc.scalar.dma_start(out=scst, in_=cstb[:, :])
            nc.sync.dma_start(out=sgb[:, 9560:BSPLIT],
                              in_=gbb[:, 9560:BSPLIT])
            nc.scalar.dma_start(out=swf[:, 0:664], in_=wfb[:, 0:664])
            nc.scalar.dma_start(out=swf[:, 664:1688], in_=wfb[:, 664:1688])
            nc.sync.dma_start(out=sgb[:, 5460:7508], in_=gbb[:, 5460:7508])
            nc.sync.dma_start(out=sgb[:, 7508:9560], in_=gbb[:, 7508:9560])
            nc.gpsimd.dma_start(out=sgb[:, 0:2730], in_=gbb[:, 0:2730])
            nc.gpsimd.dma_start(out=sgb[:, 2730:5460], in_=gbb[:, 2730:5460])



            jnk = sb.tile([1, 4], f32, tag="jnk")
            nc.scalar.copy(out=jnk, in_=scst[0:1, 0:4])

            # ---- moments ----
            psmom = pmisc.tile([128, 8], f32, tag="m1")
            for n_, c in enumerate(range(32)):
                nc.tensor.matmul(
                    psmom[:, 0:5],
                    lhsT=spta[:, 160 + c * 128:160 + (c + 1) * 128],
                    rhs=spta[:, c * 5:(c + 1) * 5],
                    start=(n_ == 0),
                    stop=(n_ == 31),
                )


            # ---- coefficient chain, Pool (g) / DVE (v) split ----
            def t(cols, tag):
                return sb.tile([128, cols], f32, tag=tag, name=tag)

            v = nc.vector
            g = v
            mom = t(5, "mom")
            v.tensor_copy(out=mom, in_=psmom[:, 0:5])  # muy mux Eyy Eyx Exx
            pyy = t(1, "pyy"); pyx_ = t(1, "pyx_"); pxx = t(1, "pxx")
            g.tensor_tensor(out=pyy, in0=mom[:, 0:1], in1=mom[:, 0:1],
                            op=AT.mult)
            g.tensor_tensor(out=pyx_, in0=mom[:, 0:1], in1=mom[:, 1:2],
                            op=AT.mult)
            g.tensor_tensor(out=pxx, in0=mom[:, 1:2], in1=mom[:, 1:2],
                            op=AT.mult)
            c00 = t(1, "c00"); c01 = t(1, "c01"); c11 = t(1, "c11")
            g.tensor_tensor(out=c00, in0=mom[:, 2:3], in1=pyy, op=AT.subtract)
            g.tensor_tensor(out=c01, in0=mom[:, 3:4], in1=pyx_,
                            op=AT.subtract)
            g.tensor_tensor(out=c11, in0=mom[:, 4:5], in1=pxx, op=AT.subtract)
            pp = t(2, "pp")
            v.tensor_scalar(out=pp, in0=mom[:, 0:2], scalar1=-1.0,
                            scalar2=EPS_DIST, op0=AT.mult, op1=AT.add)
            qyy = t(1, "qyy"); qyx = t(1, "qyx"); qxx = t(1, "qxx")
            v.tensor_tensor(out=qyy, in0=pp[:, 0:1], in1=pp[:, 0:1],
                            op=AT.mult)
            v.tensor_tensor(out=qyx, in0=pp[:, 0:1], in1=pp[:, 1:2],
                            op=AT.mult)
            v.tensor_tensor(out=qxx, in0=pp[:, 1:2], in1=pp[:, 1:2],
                            op=AT.mult)
            m1_ = t(1, "m1_"); m2_ = t(1, "m2_")
            g.tensor_tensor(out=m1_, in0=c00, in1=c11, op=AT.mult)
            g.tensor_tensor(out=m2_, in0=c01, in1=c01, op=AT.mult)
            d2 = t(1, "d2")
            g.tensor_tensor(out=d2, in0=m1_, in1=m2_, op=AT.subtract)
            d2s = t(1, "d2s")
            g.tensor_scalar(out=d2s, in0=d2,
                            scalar1=1.0 / (L_INV_SCAL * L_INV_SCAL),
                            scalar2=1e-20, op0=AT.mult, op1=AT.add)
            q = t(1, "q")
            v.reciprocal_approx_fast(out=q, in_=d2s)

            coef = sb.tile([128, 6], f32, tag="coef")
            g.tensor_tensor(out=coef[:, 3:4], in0=q, in1=c11, op=AT.mult)
            g.scalar_tensor_tensor(out=coef[:, 4:5], in0=q, scalar=-2.0,
                                   in1=c01, op0=AT.mult, op1=AT.mult)
            g.tensor_tensor(out=coef[:, 5:6], in0=q, in1=c00, op=AT.mult)
            tA = t(1, "tA"); tB = t(1, "tB"); tC = t(1, "tC")
            v.tensor_tensor(out=tA, in0=coef[:, 3:4], in1=qyy, op=AT.mult)
            v.tensor_tensor(out=tB, in0=coef[:, 4:5], in1=qyx, op=AT.mult)
            v.tensor_tensor(out=tC, in0=coef[:, 5:6], in1=qxx, op=AT.mult)
            c0a = t(1, "c0a")
            v.tensor_tensor(out=c0a, in0=tA, in1=tB, op=AT.add)
            c0b = t(1, "c0b")
            v.tensor_tensor(out=c0b, in0=c0a, in1=tC, op=AT.add)
            v.tensor_scalar_add(out=coef[:, 0:1], in0=c0b, scalar1=1.0)
            t4 = t(1, "t4"); t5 = t(1, "t5")
            g.tensor_tensor(out=t4, in0=coef[:, 3:4], in1=pp[:, 0:1],
                            op=AT.mult)
            g.tensor_tensor(out=t5, in0=coef[:, 4:5], in1=pp[:, 1:2],
                            op=AT.mult)
            g.scalar_tensor_tensor(out=coef[:, 1:2], in0=t4, scalar=2.0,
                                   in1=t5, op0=AT.mult, op1=AT.add)
            t6 = t(1, "t6"); t7 = t(1, "t7")
            g.tensor_tensor(out=t6, in0=coef[:, 4:5], in1=pp[:, 0:1],
                            op=AT.mult)
            g.tensor_tensor(out=t7, in0=coef[:, 5:6], in1=pp[:, 1:2],
                            op=AT.mult)
            g.scalar_tensor_tensor(out=coef[:, 2:3], in0=t7, scalar=2.0,
                                   in1=t6, op0=AT.mult, op1=AT.add)

            pst = pmisc.tile([6, 128], f32, tag="m1")
            nc.tensor.transpose(pst, coef, scst[:, 0:128])
            coefT = sb.tile([6, 128], bf16, tag="coefT")
            nc.vector.tensor_copy(out=coefT, in_=pst)
            # band lhsT variants: A = [coefT; 0] (legal partition-32 copy),
            # B = [0; coefT] (SBUF->SBUF DMA; latency hides under band A)
            nc.vector.tensor_copy(out=czzA[0:6, :], in_=coefT)
            nc.sync.dma_start(out=czzB[6:12, :], in_=coefT)

            # ---- heat generation ----
            heat = sb.tile([128, GB_TOT], f32, tag="heat")

            def recip_dve(off, n, ps):
                nc.vector.reciprocal_approx_fast(
                    out=heat[:, off:off + n], in_=ps)

            def recip_act(off, n, ps):
                se = nc.scalar
                se.add_instruction(
                    mybir.InstActivation(
                        name=nc.get_next_instruction_name(),
                        func=mybir.ActivationFunctionType.Reciprocal,
                        ins=[se.lower_ap(ps),
                             mybir.ImmediateValue(dtype=f32, value=0.0),
                             mybir.ImmediateValue(dtype=f32, value=1.0),
                             mybir.ImmediateValue(dtype=f32, value=0.0)],
                        outs=[se.lower_ap(heat[:, off:off + n])],
                    ))

            # ---- fmap chains (bf16), fed from an Hb bf16 copy ----
            fstate = {}
            HB0 = int(HOFF[3])

            def fm_sel(si):
                hw = HWS[3 + si]
                hb0 = int(HOFF[3 + si]) - HB0
                pss = pmisc.tile([8, 256], f32, tag="m1")
                nc.tensor.matmul(pss[:, 0:hw],
                                 lhsT=swf[:, si * 8:si * 8 + 8],
                                 rhs=fstate["Hb"][:, hb0:hb0 + hw],
                                 start=True, stop=True)
                rt = sb.tile([8, 256], f32, tag="rt", bufs=2)
                nc.vector.tensor_scalar_add(out=rt[:, 0:hw], in0=pss[:, 0:hw],
                                            scalar1=1.0)
                rr = sb.tile([8, 256], f32, tag="rr", bufs=2)
                nc.vector.reciprocal_approx_fast(out=rr[:, 0:hw],
                                                 in_=rt[:, 0:hw])
                rrb = sb.tile([8, 256], bf16, tag="rrb", bufs=2)
                nc.vector.tensor_copy(out=rrb[:, 0:hw], in_=rr[:, 0:hw])
                fstate[si] = (rrb, hw)

            def fm_rep(si):
                rrb, hw = fstate[si]
                H = heat[:, int(HOFF[3 + si]):int(HOFF[3 + si]) + hw]
                psR = pmisc.tile([128, 256], f32, tag="m1")
                nc.tensor.matmul(psR[:, 0:hw], lhsT=swf[0:8, 24:152],
                                 rhs=rrb[:, 0:hw], start=True, stop=True)
                Hn = sb.tile([128, 256], bf16, tag="Hn", bufs=2)
                nc.vector.tensor_tensor(out=Hn[:, 0:hw], in0=H,
                                        in1=psR[:, 0:hw], op=AT.mult)
                fstate[si] = (Hn, hw)

            def fm_mm(si):
                Hn, hw = fstate[si]
                fm = sb.tile([128, 4 * hw], f32, tag=f"fm{si}")
                ntl = 2 if si == 0 else 1
                for tl in range(ntl):
                    psF = pmisc.tile([128, 512], f32, tag="psF")
                    for gi in range(4 // ntl):
                        gg = tl * (4 // ntl) + gi
                        nc.tensor.matmul(
                            psF[:, gi * hw:(gi + 1) * hw],
                            lhsT=swf[:, 152 + (si * 4 + gg) * 128:
                                     152 + (si * 4 + gg + 1) * 128],
                            rhs=Hn[:, 0:hw], start=True, stop=True)
                    cols = (4 // ntl) * hw
                    if si == 0 and tl == 0:
                        nc.vector.tensor_copy(out=fm[:, 0:cols],
                                              in_=psF[:, 0:cols])
                    else:
                        nc.scalar.copy(out=fm[:, tl * cols:(tl + 1) * cols],
                                       in_=psF[:, 0:cols])
                fstate[100 + si] = fm

            def hb_copy():
                Hb = sb.tile([128, 336], bf16, tag="Hb")
                nc.vector.tensor_copy(out=Hb, in_=heat[:, HOFF[3]:GB_TOT])
                fstate["Hb"] = Hb

            events = {
                1: lambda: hb_copy(),
                2: lambda: (fm_sel(0),
                            nc.gpsimd.dma_start(out=oreg(3),
                                                in_=heat[:, HOFF[3]:GB_TOT])),
                3: lambda: (fm_rep(0),
                            nc.gpsimd.dma_start(out=oreg(2),
                                                in_=heat[:, HOFF[2]:HOFF[3]])),
                4: lambda: fm_mm(0),
                5: lambda: fm_sel(1),
                6: lambda: fm_rep(1),
                7: lambda: fm_mm(1),
                8: lambda: fm_sel(2),
                9: lambda: fm_rep(2),
                10: lambda: fm_mm(2),
                11: lambda: (
                    nc.gpsimd.dma_start(out=oreg(4), in_=fstate[100]),
                    nc.gpsimd.dma_start(out=oreg(5), in_=fstate[101]),
                    nc.gpsimd.dma_start(out=oreg(1),
                                        in_=heat[:, HOFF[1]:HOFF[2]])),
                12: lambda: nc.gpsimd.dma_start(out=oreg(6), in_=fstate[102]),
            }

            ntile = len(GEN_TILES)
            for ti, (off, n, band) in enumerate(GEN_TILES):
                ps = pgen.tile([128, 512], f32, tag="ps")
                lhsT = czzA if band == 0 else czzB
                gc = off - BSPLIT if band == 0 else off
                nc.tensor.matmul(ps[:, 0:n], lhsT=lhsT,
                                 rhs=sgb[:, gc:gc + n],
                                 start=True, stop=True)
                if ti == ntile - 1:
                    recip_dve(off, n, ps[:, 0:n])
                else:
                    PAT11 = (recip_dve, recip_act, recip_dve, recip_act,
                             recip_dve, recip_act, recip_act, recip_dve,
                             recip_act, recip_dve, recip_act)
                    PAT11[ti % 11](off, n, ps[:, 0:n])
                if ti in events:
                    events[ti]()

            # stage-0 part heat: everything below heat col 16384 done last
            nc.sync.dma_start(out=oreg(0), in_=heat[:, HOFF[0]:HOFF[1]])
    nc.compile()
    return nc


def _get_nc():
    if "nc" not in _NC_CACHE:
        _NC_CACHE["nc"] = _build()
    return _NC_CACHE["nc"]


def _in_maps(part_maps, features):
    part_maps = np.asarray(part_maps, dtype=np.float32)
    features = np.asarray(features, dtype=np.float32)
    gb12, mb, cst, selb, repb = _host_consts()
    in_maps = []
    for core in range(NCORES):
        pm = part_maps[core * BL:(core + 1) * BL]
        pmr = pm.transpose(1, 0, 2, 3).reshape(ROWS, HMAP * HMAP)
        ptm = pmr.reshape(128, 32, 128).transpose(2, 1, 0).reshape(128, 4096)
        pta = np.concatenate([mb, ptm], axis=1).astype(np.float32)
        pta = np.ascontiguousarray(pta)
        wf = _host_wf(features[core * BL:(core + 1) * BL])
        wfp = np.concatenate([selb, repb, wf], axis=1)
        in_maps.append({"pta": pta, "gbb": gb12, "cstb": cst, "wfb": wfp})
    return in_maps


def _unpack(arr):
    """arr: [OUT_ROWS, STP] padded f32 -> (BL, OUT_TOT) for one core."""
    out = np.empty((BL, OUT_TOT), dtype=np.float32)

    def reg(ri, shape):
        r0, r1 = int(REGION_ROFF[ri]), int(REGION_ROFF[ri + 1])
        return arr[r0:r1, 0:REGION_W[ri]].reshape(shape)

    for s in (0, 1, 2):
        hw = HWS[s]
        block = reg(s, (NK, BL, hw))
        out[:, OUT_PH[s]:OUT_PH[s] + NK * hw] = \
            block.transpose(1, 0, 2).reshape(BL, NK * hw)
    b345 = reg(3, (128, 336))
    for s, c0 in ((3, 0), (4, 256), (5, 320)):
        pd = STAGES[s][2]
        hw = HWS[s]
        blk = b345[0:pd * BL, c0:c0 + hw].reshape(pd, BL, hw)
        out[:, OUT_PH[s]:OUT_PH[s] + pd * hw] = \
            blk.transpose(1, 0, 2).reshape(BL, pd * hw)
    for si, sidx in enumerate((3, 4, 5)):
        hw = HWS[sidx]
        block = reg(4 + si, (128, 4 * hw))
        fb = block.reshape(2, NF, 4, hw)             # (bo, n, g, f)
        fmap = fb.transpose(2, 0, 1, 3).reshape(BL, NF * hw)
        out[:, OUT_FM[sidx]:OUT_FM[sidx] + NF * hw] = fmap
    return out


def _run(part_maps, features, trace=False):
    from concourse.bass_utils import run_bass_kernel_spmd
    nc = _get_nc()
    res = run_bass_kernel_spmd(nc, _in_maps(part_maps, features),
                               list(range(NCORES)), trace=trace)
    outs = [_unpack(res.results[i]["out"]) for i in range(NCORES)]
    return np.concatenate(outs, axis=0), res


def kernel(part_maps, features):
    out, _ = _run(part_maps, features, trace=False)
    return out


# revision 19
# speedup vs baseline: 5.1124x; 1.0150x over previous
"""Trainium2 Bass kernel v4 for the part-map heatmap-pyramid encoder.

Contract: kernel(part_maps, features) -> (64, 369952) float32.
Data parallel over batch: 8 samples per NeuronCore x 8 cores.

Per-core pipeline:
  1. moments: 32 accumulating fp32 matmuls over pixel chunks -> [128, 5];
     moment basis is folded into the same DRAM load as the part maps.
  2. coefficient chain split over Pool+DVE (no sqrt: L_inv^T L_inv =
     (0.8/det)^2 [[s11,-s01],[-s01,s00]]), one divide.
  3. transpose coef -> [6, 128] bf16.
  4. gen: rank-6 bf16 matmuls (PE) from a resident [6, 21840] bf16 basis
     into [128, 1024] PSUM tiles; heat = 1/proj via Pool divide / DVE
     reciprocal_approx_fast; all heat in one [128, 21840] f32 SBUF tile.
  5. fmap chains for stages 3-5, interleaved into the gen stream.
  6. stores: padded flat DRAM regions (500ns each in-model), host unpack.
"""

import numpy as np

BN, NK, NF, HMAP = 64, 16, 64, 64
NCORES = 8
BL = BN // NCORES            # samples per core = 8
ROWS = BL * NK               # partition rows per core = 128
L_INV_SCAL = 0.8
EPS_DIST = 1e-6
EPS_COV = 1e-12

STAGES = [(128, 128, NK, (0, 0)), (64, 64, NK, (0, 0)), (32, 32, NK, (0, 0)),
          (16, 16, NK, (4, NK)), (8, 8, 4, (2, 4)), (4, 4, 2, (0, 2))]
HWS = [h * w for (h, w, _, _) in STAGES]
HOFF = np.concatenate([[0], np.cumsum(HWS)]).astype(int)
GB_TOT = int(HOFF[-1])       # 21840

_off = 0
OUT_PH = []
OUT_FM = []
for (h, w, pd, (s0, s1)) in STAGES:
    OUT_PH.append(_off)
    _off += pd * h * w
    if s1 - s0 != 0:
        OUT_FM.append(_off)
        _off += NF * h * w
    else:
        OUT_FM.append(None)
OUT_TOT = _off               # 369952

# ---- padded output regions: [elems/64 rows, 80] each (64 data cols) ----
STP = 80
# R0, R1, R2 (stage 0-2 part heats), R345 (stages 3-5 cols incl garbage
# rows), F3, F4, F5 (fmap tiles)
REGION_ELEMS = [128 * HWS[0], 128 * HWS[1], 128 * HWS[2], 128 * 336,
                128 * 4 * HWS[3], 128 * 4 * HWS[4], 128 * 4 * HWS[5]]
REGION_W = [64, 64, 64, 16, 64, 64, 64]
REGION_ROWS = [e // w for e, w in zip(REGION_ELEMS, REGION_W)]
REGION_ROFF = np.concatenate([[0], np.cumsum(REGION_ROWS)]).astype(int)
OUT_ROWS = int(REGION_ROFF[-1])

# 2-band basis split point (heat cols >= BSPLIT live in sgb12 rows 0-5)
BSPLIT = 10920
# gen tiles: (heat_offset, ncols, band): band A ordered stage345 tail,
# stage2, stage1, stage0-back; then band B = stage0 front
GEN_TILES = [(int(HOFF[3]), 336, 0)]
for c in range(int(HOFF[2]), int(HOFF[3]), 512):
    GEN_TILES.append((c, 512, 0))
for c in range(int(HOFF[1]), int(HOFF[2]), 512):
    GEN_TILES.append((c, 512, 0))
for c in range(BSPLIT, int(HOFF[1]), 512):
    GEN_TILES.append((c, min(512, int(HOFF[1]) - c), 0))
for c in range(0, BSPLIT, 512):
    GEN_TILES.append((c, min(512, BSPLIT - c), 1))


def _mesh_basis(h, w):
    y = np.linspace(-1.0, 1.0, h, dtype=np.float64)
    x = np.linspace(-1.0, 1.0, w, dtype=np.float64)
    yy = np.repeat(y, w)
    xx = np.tile(x, h)
    return np.stack([np.ones_like(yy), yy, xx, yy * yy, yy * xx, xx * xx])


def _bf16(a):
    import ml_dtypes
    return np.asarray(a, dtype=np.float32).astype(ml_dtypes.bfloat16)


def _host_consts():
    gb = np.concatenate([_mesh_basis(h, w) for (h, w, _, _) in STAGES], axis=1)
    # 2-band layout [12, BSPLIT]: rows 0-5 = cols BSPLIT.., rows 6-11 = front
    gb12 = np.concatenate([gb[:, BSPLIT:], gb[:, 0:BSPLIT]], axis=0)
    bm = _mesh_basis(HMAP, HMAP)[1:6]              # [5, 4096]
    mb = np.zeros((128, 160), dtype=np.float32)
    for c in range(32):
        mb[:, c * 5:(c + 1) * 5] = bm[:, c * 128:(c + 1) * 128].T
    cst = np.eye(128, dtype=np.float32)
    sel = np.zeros((128, 24), dtype=np.float32)
    rep = np.zeros((128, 128), dtype=np.float32)
    for b in range(BL):
        for k in range(NK):
            rep[b, k * 8 + b] = 1.0
        for si, sidx in enumerate((3, 4, 5)):
            s0, s1 = STAGES[sidx][3]
            for k in range(s0, s1):
                sel[k * 8 + b, si * 8 + b] = 1.0
    return _bf16(gb12), mb, cst, _bf16(sel), _bf16(rep)


def _host_wf(features_core):
    wf = np.zeros((128, 12 * 128), dtype=np.float32)
    for si, sidx in enumerate((3, 4, 5)):
        s0, s1 = STAGES[sidx][3]
        for g in range(4):
            blk = (si * 4 + g) * 128
            for bo in range(2):
                b = 2 * g + bo
                for k in range(s0, s1):
                    wf[k * 8 + b, blk + 64 * bo:blk + 64 * (bo + 1)] = \
                        features_core[b, k, :]
    return _bf16(wf)


_NC_CACHE = {}


def _build():
    import concourse.bass as bass
    import concourse.bacc as bacc
    import concourse.tile as tile
    from concourse import mybir

    f32 = mybir.dt.float32
    bf16 = mybir.dt.bfloat16
    AT = mybir.AluOpType

    nc = bacc.Bacc("TRN2", target_bir_lowering=False, debug=False)
    # pta: [mb(160) | pt(4096)] f32
    pta = nc.declare_dram_parameter("pta", [128, 4256], f32, isOutput=False)
    gbb = nc.declare_dram_parameter("gbb", [12, BSPLIT], bf16, isOutput=False)
    cstb = nc.declare_dram_parameter("cstb", [128, 128], f32, isOutput=False)
    wfb = nc.declare_dram_parameter("wfb", [128, 1688], bf16, isOutput=False)
    outp = nc.declare_dram_parameter("out", [OUT_ROWS, STP], f32,
                                     isOutput=True)

    def oreg(ri):
        r0, r1 = int(REGION_ROFF[ri]), int(REGION_ROFF[ri + 1])
        return outp[r0:r1, 0:REGION_W[ri]]

    with tile.TileContext(nc) as tc:
        import contextlib
        ctx = contextlib.ExitStack()
        with ctx:
            sb = ctx.enter_context(tc.tile_pool(name="sb", bufs=1))
            pgen = ctx.enter_context(tc.tile_pool(name="pgen", bufs=6,
                                                  space="PSUM"))
            pmisc = ctx.enter_context(tc.tile_pool(name="pmisc", bufs=1,
                                                   space="PSUM"))

            czzA = sb.tile([12, 128], bf16, tag="czzA")
            nc.gpsimd.memset(czzA, 0.0)
            czzB = sb.tile([12, 128], bf16, tag="czzB")
            nc.gpsimd.memset(czzB, 0.0)
            spta = sb.tile([128, 4256], f32, tag="spta")
            scst = sb.tile([128, 128], f32, tag="scst")
            sgb = sb.tile([12, BSPLIT], bf16, tag="sgb")
            swf = sb.tile([128, 1688], bf16, tag="swf")

            # pt pieces: [0:1184]=mb+chunks0-7, then 1024-col pieces
            nc.sync.dma_start(out=spta[:, 0:1184], in_=pta[:, 0:1184])
            nc.scalar.dma_start(out=spta[:, 1184:2208], in_=pta[:, 1184:2208])
            nc.gpsimd.dma_start(out=spta[:, 2208:3232], in_=pta[:, 2208:3232])
            nc.gpsimd.dma_start(out=spta[:, 3232:4256], in_=pta[:, 3232:4256])
            # basis on SP in gen-consumption order (high cols first);
            # Act only gets small early loads so its recip window is clear
            nc.scalar.dma_start(out=scst, in_=cstb[:, :])
            nc.sync.dma_start(out=sgb[:, 9560:BSPLIT],
                              in_=gbb[:, 9560:BSPLIT])
            nc.scalar.dma_start(out=swf[:, 0:664], in_=wfb[:, 0:664])
            nc.scalar.dma_start(out=swf[:, 664:1688], in_=wfb[:, 664:1688])
            nc.sync.dma_start(out=sgb[:, 5460:7508], in_=gbb[:, 5460:7508])
            nc.sync.dma_start(out=sgb[:, 7508:9560], in_=gbb[:, 7508:9560])
            nc.gpsimd.dma_start(out=sgb[:, 0:2730], in_=gbb[:, 0:2730])
            nc.gpsimd.dma_start(out=sgb[:, 2730:5460], in_=gbb[:, 2730:5460])



            jnk = sb.tile([1, 4], f32, tag="jnk")
            nc.scalar.copy(out=jnk, in_=scst[0:1, 0:4])

            # ---- moments ----
            psmom = pmisc.tile([128, 8], f32, tag="m1")
            for n_, c in enumerate(range(32)):
                nc.tensor.matmul(
                    psmom[:, 0:5],
                    lhsT=spta[:, 160 + c * 128:160 + (c + 1) * 128],
                    rhs=spta[:, c * 5:(c + 1) * 5],
                    start=(n_ == 0),
                    stop=(n_ == 31),
                )


            # ---- coefficient chain, Pool (g) / DVE (v) split ----
            def t(cols, tag):
                return sb.tile([128, cols], f32, tag=tag, name=tag)

            v = nc.vector
            g = v
            mom = t(5, "mom")
            v.tensor_copy(out=mom, in_=psmom[:, 0:5])  # muy mux Eyy Eyx Exx
            pyy = t(1, "pyy"); pyx_ = t(1, "pyx_"); pxx = t(1, "pxx")
            g.tensor_tensor(out=pyy, in0=mom[:, 0:1], in1=mom[:, 0:1],
                            op=AT.mult)
            g.tensor_tensor(out=pyx_, in0=mom[:, 0:1], in1=mom[:, 1:2],
                            op=AT.mult)
            g.tensor_tensor(out=pxx, in0=mom[:, 1:2], in1=mom[:, 1:2],
                            op=AT.mult)
            c00 = t(1, "c00"); c01 = t(1, "c01"); c11 = t(1, "c11")
            g.tensor_tensor(out=c00, in0=mom[:, 2:3], in1=pyy, op=AT.subtract)
            g.tensor_tensor(out=c01, in0=mom[:, 3:4], in1=pyx_,
                            op=AT.subtract)
            g.tensor_tensor(out=c11, in0=mom[:, 4:5], in1=pxx, op=AT.subtract)
            pp = t(2, "pp")
            v.tensor_scalar(out=pp, in0=mom[:, 0:2], scalar1=-1.0,
                            scalar2=EPS_DIST, op0=AT.mult, op1=AT.add)
            qyy = t(1, "qyy"); qyx = t(1, "qyx"); qxx = t(1, "qxx")
            v.tensor_tensor(out=qyy, in0=pp[:, 0:1], in1=pp[:, 0:1],
                            op=AT.mult)
            v.tensor_tensor(out=qyx, in0=pp[:, 0:1], in1=pp[:, 1:2],
                            op=AT.mult)
            v.tensor_tensor(out=qxx, in0=pp[:, 1:2], in1=pp[:, 1:2],
                            op=AT.mult)
            m1_ = t(1, "m1_"); m2_ = t(1, "m2_")
            g.tensor_tensor(out=m1_, in0=c00, in1=c11, op=AT.mult)
            g.tensor_tensor(out=m2_, in0=c01, in1=c01, op=AT.mult)
            d2 = t(1, "d2")
            g.tensor_tensor(out=d2, in0=m1_, in1=m2_, op=AT.subtract)
            d2s = t(1, "d2s")
            g.tensor_scalar(out=d2s, in0=d2,
                            scalar1=1.0 / (L_INV_SCAL * L_INV_SCAL),
                            scalar2=1e-20, op0=AT.mult, op1=AT.add)
            q = t(1, "q")
            v.reciprocal_approx_fast(out=q, in_=d2s)

            coef = sb.tile([128, 6], f32, tag="coef")
            g.tensor_tensor(out=coef[:, 3:4], in0=q, in1=c11, op=AT.mult)
            g.scalar_tensor_tensor(out=coef[:, 4:5], in0=q, scalar=-2.0,
                                   in1=c01, op0=AT.mult, op1=AT.mult)
            g.tensor_tensor(out=coef[:, 5:6], in0=q, in1=c00, op=AT.mult)
            tA = t(1, "tA"); tB = t(1, "tB"); tC = t(1, "tC")
            v.tensor_tensor(out=tA, in0=coef[:, 3:4], in1=qyy, op=AT.mult)
            v.tensor_tensor(out=tB, in0=coef[:, 4:5], in1=qyx, op=AT.mult)
            v.tensor_tensor(out=tC, in0=coef[:, 5:6], in1=qxx, op=AT.mult)
            c0a = t(1, "c0a")
            v.tensor_tensor(out=c0a, in0=tA, in1=tB, op=AT.add)
            c0b = t(1, "c0b")
            v.tensor_tensor(out=c0b, in0=c0a, in1=tC, op=AT.add)
            v.tensor_scalar_add(out=coef[:, 0:1], in0=c0b, scalar1=1.0)
            t4 = t(1, "t4"); t5 = t(1, "t5")
            g.tensor_tensor(out=t4, in0=coef[:, 3:4], in1=pp[:, 0:1],
                            op=AT.mult)
            g.tensor_tensor(out=t5, in0=coef[:, 4:5], in1=pp[:, 1:2],
                            op=AT.mult)
            g.scalar_tensor_tensor(out=coef[:, 1:2], in0=t4, scalar=2.0,
                                   in1=t5, op0=AT.mult, op1=AT.add)
            t6 = t(1, "t6"); t7 = t(1, "t7")
            g.tensor_tensor(out=t6, in0=coef[:, 4:5], in1=pp[:, 0:1],
                            op=AT.mult)
            g.tensor_tensor(out=t7, in0=coef[:, 5:6], in1=pp[:, 1:2],
                            op=AT.mult)
            g.scalar_tensor_tensor(out=coef[:, 2:3], in0=t7, scalar=2.0,
                                   in1=t6, op0=AT.mult, op1=AT.add)

            pst = pmisc.tile([6, 128], f32, tag="m1")
            nc.tensor.transpose(pst, coef, scst[:, 0:128])
            coefT = sb.tile([6, 128], bf16, tag="coefT")
            nc.vector.tensor_copy(out=coefT, in_=pst)
            # band lhsT variants: A = [coefT; 0] (legal partition-32 copy),
            # B = [0; coefT] (SBUF->SBUF DMA; latency hides under band A)
            nc.vector.tensor_copy(out=czzA[0:6, :], in_=coefT)
            nc.sync.dma_start(out=czzB[6:12, :], in_=coefT)

            # ---- heat generation ----
            heat = sb.tile([128, GB_TOT], f32, tag="heat")

            def recip_dve(off, n, ps):
                nc.vector.reciprocal_approx_fast(
                    out=heat[:, off:off + n], in_=ps)

            def recip_act(off, n, ps):
                se = nc.scalar
                se.add_instruction(
                    mybir.InstActivation(
                        name=nc.get_next_instruction_name(),
                        func=mybir.ActivationFunctionType.Reciprocal,
                        ins=[se.lower_ap(ps),
                             mybir.ImmediateValue(dtype=f32, value=0.0),
                             mybir.ImmediateValue(dtype=f32, value=1.0),
                             mybir.ImmediateValue(dtype=f32, value=0.0)],
                        outs=[se.lower_ap(heat[:, off:off + n])],
                    ))

            # ---- fmap chains (bf16), fed from an Hb bf16 copy ----
            fstate = {}
            HB0 = int(HOFF[3])

            def fm_sel(si):
                hw = HWS[3 + si]
                hb0 = int(HOFF[3 + si]) - HB0
                pss = pmisc.tile([8, 256], f32, tag="m1")
                nc.tensor.matmul(pss[:, 0:hw],
                                 lhsT=swf[:, si * 8:si * 8 + 8],
                                 rhs=fstate["Hb"][:, hb0:hb0 + hw],
                                 start=True, stop=True)
                rt = sb.tile([8, 256], f32, tag="rt", bufs=2)
                nc.vector.tensor_scalar_add(out=rt[:, 0:hw], in0=pss[:, 0:hw],
                                            scalar1=1.0)
                rr = sb.tile([8, 256], f32, tag="rr", bufs=2)
                nc.vector.reciprocal_approx_fast(out=rr[:, 0:hw],
                                                 in_=rt[:, 0:hw])
                rrb = sb.tile([8, 256], bf16, tag="rrb", bufs=2)
                nc.vector.tensor_copy(out=rrb[:, 0:hw], in_=rr[:, 0:hw])
                fstate[si] = (rrb, hw)

            def fm_rep(si):
                rrb, hw = fstate[si]
                H = heat[:, int(HOFF[3 + si]):int(HOFF[3 + si]) + hw]
                psR = pmisc.tile([128, 256], f32, tag="m1")
                nc.tensor.matmul(psR[:, 0:hw], lhsT=swf[0:8, 24:152],
                                 rhs=rrb[:, 0:hw], start=True, stop=True)
                Hn = sb.tile([128, 256], bf16, tag="Hn", bufs=2)
                nc.vector.tensor_tensor(out=Hn[:, 0:hw], in0=H,
                                        in1=psR[:, 0:hw], op=AT.mult)
                fstate[si] = (Hn, hw)

            def fm_mm(si):
                Hn, hw = fstate[si]
                fm = sb.tile([128, 4 * hw], f32, tag=f"fm{si}")
                ntl = 2 if si == 0 else 1
                for tl in range(ntl):
                    psF = pmisc.tile([128, 512], f32, tag="psF")
                    for gi in range(4 // ntl):
                        gg = tl * (4 // ntl) + gi
                        nc.tensor.matmul(
                            psF[:, gi * hw:(gi + 1) * hw],
                            lhsT=swf[:, 152 + (si * 4 + gg) * 128:
                                     152 + (si * 4 + gg + 1) * 128],
                            rhs=Hn[:, 0:hw], start=True, stop=True)
                    cols = (4 // ntl) * hw
                    if si == 0 and tl == 0:
                        nc.vector.tensor_copy(out=fm[:, 0:cols],
                                              in_=psF[:, 0:cols])
                    else:
                        nc.scalar.copy(out=fm[:, tl * cols:(tl + 1) * cols],
                                       in_=psF[:, 0:cols])
                fstate[100 + si] = fm

            def hb_copy():
                Hb = sb.tile([128, 336], bf16, tag="Hb")
                nc.vector.tensor_copy(out=Hb, in_=heat[:, HOFF[3]:GB_TOT])
                fstate["Hb"] = Hb

            events = {
                1: lambda: hb_copy(),
                2: lambda: (fm_sel(0),
                            nc.gpsimd.dma_start(out=oreg(3),
                                                in_=heat[:, HOFF[3]:GB_TOT])),
                3: lambda: (fm_rep(0),
                            nc.gpsimd.dma_start(out=oreg(2),
                                                in_=heat[:, HOFF[2]:HOFF[3]])),
                4: lambda: fm_mm(0),
                5: lambda: fm_sel(1),
                6: lambda: fm_rep(1),
                7: lambda: fm_mm(1),
                8: lambda: fm_sel(2),
                9: lambda: fm_rep(2),
                10: lambda: fm_mm(2),
                11: lambda: (
                    nc.gpsimd.dma_start(out=oreg(4), in_=fstate[100]),
                    nc.gpsimd.dma_start(out=oreg(5), in_=fstate[101]),
                    nc.gpsimd.dma_start(out=oreg(1),
                                        in_=heat[:, HOFF[1]:HOFF[2]])),
                12: lambda: nc.gpsimd.dma_start(out=oreg(6), in_=fstate[102]),
            }

            ntile = len(GEN_TILES)
            for ti, (off, n, band) in enumerate(GEN_TILES):
                ps = pgen.tile([128, 512], f32, tag="ps")
                lhsT = czzA if band == 0 else czzB
                gc = off - BSPLIT if band == 0 else off
                nc.tensor.matmul(ps[:, 0:n], lhsT=lhsT,
                                 rhs=sgb[:, gc:gc + n],
                                 start=True, stop=True)
                if ti == ntile - 1:
                    recip_dve(off, n, ps[:, 0:n])
                elif ti == 40:
                    recip_act(off, n, ps[:, 0:n])
                else:
                    PAT11 = (recip_dve, recip_act, recip_dve, recip_act,
                             recip_dve, recip_act, recip_act, recip_dve,
                             recip_act, recip_dve, recip_act)
                    PAT11[ti % 11](off, n, ps[:, 0:n])
                if ti in events:
                    events[ti]()

            # stage-0 part heat: everything below heat col 16384 done last
            nc.sync.dma_start(out=oreg(0), in_=heat[:, HOFF[0]:HOFF[1]])
    nc.compile()
    return nc


def _get_nc():
    if "nc" not in _NC_CACHE:
        _NC_CACHE["nc"] = _build()
    return _NC_CACHE["nc"]


def _in_maps(part_maps, features):
    part_maps = np.asarray(part_maps, dtype=np.float32)
    features = np.asarray(features, dtype=np.float32)
    gb12, mb, cst, selb, repb = _host_consts()
    in_maps = []
    for core in range(NCORES):
        pm = part_maps[core * BL:(core + 1) * BL]
        pmr = pm.transpose(1, 0, 2, 3).reshape(ROWS, HMAP * HMAP)
        ptm = pmr.reshape(128, 32, 128).transpose(2, 1, 0).reshape(128, 4096)
        pta = np.concatenate([mb, ptm], axis=1).astype(np.float32)
        pta = np.ascontiguousarray(pta)
        wf = _host_wf(features[core * BL:(core + 1) * BL])
        wfp = np.concatenate([selb, repb, wf], axis=1)
        in_maps.append({"pta": pta, "gbb": gb12, "cstb": cst, "wfb": wfp})
    return in_maps


def _unpack(arr):
    """arr: [OUT_ROWS, STP] padded f32 -> (BL, OUT_TOT) for one core."""
    out = np.empty((BL, OUT_TOT), dtype=np.float32)

    def reg(ri, shape):
        r0, r1 = int(REGION_ROFF[ri]), int(REGION_ROFF[ri + 1])
        return arr[r0:r1, 0:REGION_W[ri]].reshape(shape)

    for s in (0, 1, 2):
        hw = HWS[s]
        block = reg(s, (NK, BL, hw))
        out[:, OUT_PH[s]:OUT_PH[s] + NK * hw] = \
            block.transpose(1, 0, 2).reshape(BL, NK * hw)
    b345 = reg(3, (128, 336))
    for s, c0 in ((3, 0), (4, 256), (5, 320)):
        pd = STAGES[s][2]
        hw = HWS[s]
        blk = b345[0:pd * BL, c0:c0 + hw].reshape(pd, BL, hw)
        out[:, OUT_PH[s]:OUT_PH[s] + pd * hw] = \
            blk.transpose(1, 0, 2).reshape(BL, pd * hw)
    for si, sidx in enumerate((3, 4, 5)):
        hw = HWS[sidx]
        block = reg(4 + si, (128, 4 * hw))
        fb = block.reshape(2, NF, 4, hw)             # (bo, n, g, f)
        fmap = fb.transpose(2, 0, 1, 3).reshape(BL, NF * hw)
        out[:, OUT_FM[sidx]:OUT_FM[sidx] + NF * hw] = fmap
    return out


def _run(part_maps, features, trace=False):
    from concourse.bass_utils import run_bass_kernel_spmd
    nc = _get_nc()
    res = run_bass_kernel_spmd(nc, _in_maps(part_maps, features),
                               list(range(NCORES)), trace=trace)
    outs = [_unpack(res.results[i]["out"]) for i in range(NCORES)]
    return np.concatenate(outs, axis=0), res


def kernel(part_maps, features):
    out, _ = _run(part_maps, features, trace=False)
    return out


# revision 24
# speedup vs baseline: 5.1222x; 1.0019x over previous
"""Trainium2 Bass kernel v4 for the part-map heatmap-pyramid encoder.

Contract: kernel(part_maps, features) -> (64, 369952) float32.
Data parallel over batch: 8 samples per NeuronCore x 8 cores.

Per-core pipeline:
  1. moments: 32 accumulating fp32 matmuls over pixel chunks -> [128, 5];
     moment basis is folded into the same DRAM load as the part maps.
  2. coefficient chain split over Pool+DVE (no sqrt: L_inv^T L_inv =
     (0.8/det)^2 [[s11,-s01],[-s01,s00]]), one divide.
  3. transpose coef -> [6, 128] bf16.
  4. gen: rank-6 bf16 matmuls (PE) from a resident [6, 21840] bf16 basis
     into [128, 1024] PSUM tiles; heat = 1/proj via Pool divide / DVE
     reciprocal_approx_fast; all heat in one [128, 21840] f32 SBUF tile.
  5. fmap chains for stages 3-5, interleaved into the gen stream.
  6. stores: padded flat DRAM regions (500ns each in-model), host unpack.
"""

import numpy as np

BN, NK, NF, HMAP = 64, 16, 64, 64
NCORES = 8
BL = BN // NCORES            # samples per core = 8
ROWS = BL * NK               # partition rows per core = 128
L_INV_SCAL = 0.8
EPS_DIST = 1e-6
EPS_COV = 1e-12

STAGES = [(128, 128, NK, (0, 0)), (64, 64, NK, (0, 0)), (32, 32, NK, (0, 0)),
          (16, 16, NK, (4, NK)), (8, 8, 4, (2, 4)), (4, 4, 2, (0, 2))]
HWS = [h * w for (h, w, _, _) in STAGES]
HOFF = np.concatenate([[0], np.cumsum(HWS)]).astype(int)
GB_TOT = int(HOFF[-1])       # 21840

_off = 0
OUT_PH = []
OUT_FM = []
for (h, w, pd, (s0, s1)) in STAGES:
    OUT_PH.append(_off)
    _off += pd * h * w
    if s1 - s0 != 0:
        OUT_FM.append(_off)
        _off += NF * h * w
    else:
        OUT_FM.append(None)
OUT_TOT = _off               # 369952

# ---- padded output regions: [elems/64 rows, 80] each (64 data cols) ----
STP = 80
# R0, R1, R2 (stage 0-2 part heats), R345 (stages 3-5 cols incl garbage
# rows), F3, F4, F5 (fmap tiles)
REGION_ELEMS = [128 * HWS[0], 128 * HWS[1], 128 * HWS[2], 128 * 336,
                128 * 4 * HWS[3], 128 * 4 * HWS[4], 128 * 4 * HWS[5]]
REGION_W = [64, 64, 64, 16, 64, 64, 64]
REGION_ROWS = [e // w for e, w in zip(REGION_ELEMS, REGION_W)]
REGION_ROFF = np.concatenate([[0], np.cumsum(REGION_ROWS)]).astype(int)
OUT_ROWS = int(REGION_ROFF[-1])

# 2-band basis split point (heat cols >= BSPLIT live in sgb12 rows 0-5)
BSPLIT = 10920
# gen tiles: (heat_offset, ncols, band): band A ordered stage345 tail,
# stage2, stage1, stage0-back; then band B = stage0 front
GEN_TILES = [(int(HOFF[3]), 336, 0)]
for c in range(int(HOFF[2]), int(HOFF[3]), 512):
    GEN_TILES.append((c, 512, 0))
for c in range(int(HOFF[1]), int(HOFF[2]), 512):
    GEN_TILES.append((c, 512, 0))
for c in range(BSPLIT, int(HOFF[1]), 512):
    GEN_TILES.append((c, min(512, int(HOFF[1]) - c), 0))
for c in range(0, BSPLIT, 512):
    GEN_TILES.append((c, min(512, BSPLIT - c), 1))


def _mesh_basis(h, w):
    y = np.linspace(-1.0, 1.0, h, dtype=np.float64)
    x = np.linspace(-1.0, 1.0, w, dtype=np.float64)
    yy = np.repeat(y, w)
    xx = np.tile(x, h)
    return np.stack([np.ones_like(yy), yy, xx, yy * yy, yy * xx, xx * xx])


def _bf16(a):
    import ml_dtypes
    return np.asarray(a, dtype=np.float32).astype(ml_dtypes.bfloat16)


def _host_consts():
    gb = np.concatenate([_mesh_basis(h, w) for (h, w, _, _) in STAGES], axis=1)
    # 2-band layout [12, BSPLIT]: rows 0-5 = cols BSPLIT.., rows 6-11 = front
    gb12 = np.concatenate([gb[:, BSPLIT:], gb[:, 0:BSPLIT]], axis=0)
    bm = _mesh_basis(HMAP, HMAP)[1:6]              # [5, 4096]
    mb = np.zeros((128, 160), dtype=np.float32)
    for c in range(32):
        mb[:, c * 5:(c + 1) * 5] = bm[:, c * 128:(c + 1) * 128].T
    cst = np.eye(128, dtype=np.float32)
    sel = np.zeros((128, 24), dtype=np.float32)
    rep = np.zeros((128, 128), dtype=np.float32)
    for b in range(BL):
        for k in range(NK):
            rep[b, k * 8 + b] = 1.0
        for si, sidx in enumerate((3, 4, 5)):
            s0, s1 = STAGES[sidx][3]
            for k in range(s0, s1):
                sel[k * 8 + b, si * 8 + b] = 1.0
    return _bf16(gb12), mb, cst, _bf16(sel), _bf16(rep)


def _host_wf(features_core):
    wf = np.zeros((128, 12 * 128), dtype=np.float32)
    for si, sidx in enumerate((3, 4, 5)):
        s0, s1 = STAGES[sidx][3]
        for g in range(4):
            blk = (si * 4 + g) * 128
            for bo in range(2):
                b = 2 * g + bo
                for k in range(s0, s1):
                    wf[k * 8 + b, blk + 64 * bo:blk + 64 * (bo + 1)] = \
                        features_core[b, k, :]
    return _bf16(wf)


_NC_CACHE = {}


def _build():
    import concourse.bass as bass
    import concourse.bacc as bacc
    import concourse.tile as tile
    from concourse import mybir

    f32 = mybir.dt.float32
    bf16 = mybir.dt.bfloat16
    AT = mybir.AluOpType

    nc = bacc.Bacc("TRN2", target_bir_lowering=False, debug=False)
    # pta: [mb(160) | pt(4096)] f32
    pta = nc.declare_dram_parameter("pta", [128, 4256], f32, isOutput=False)
    gbb = nc.declare_dram_parameter("gbb", [12, BSPLIT], bf16, isOutput=False)
    cstb = nc.declare_dram_parameter("cstb", [128, 128], f32, isOutput=False)
    wfb = nc.declare_dram_parameter("wfb", [128, 1688], bf16, isOutput=False)
    outp = nc.declare_dram_parameter("out", [OUT_ROWS, STP], f32,
                                     isOutput=True)

    def oreg(ri):
        r0, r1 = int(REGION_ROFF[ri]), int(REGION_ROFF[ri + 1])
        return outp[r0:r1, 0:REGION_W[ri]]

    with tile.TileContext(nc) as tc:
        import contextlib
        ctx = contextlib.ExitStack()
        with ctx:
            sb = ctx.enter_context(tc.tile_pool(name="sb", bufs=1))
            pgen = ctx.enter_context(tc.tile_pool(name="pgen", bufs=6,
                                                  space="PSUM"))
            pmisc = ctx.enter_context(tc.tile_pool(name="pmisc", bufs=1,
                                                   space="PSUM"))

            czzA = sb.tile([12, 128], bf16, tag="czzA")
            nc.gpsimd.memset(czzA, 0.0)
            czzB = sb.tile([12, 128], bf16, tag="czzB")
            nc.gpsimd.memset(czzB, 0.0)
            spta = sb.tile([128, 4256], f32, tag="spta")
            scst = sb.tile([128, 128], f32, tag="scst")
            sgb = sb.tile([12, BSPLIT], bf16, tag="sgb")
            swf = sb.tile([128, 1688], bf16, tag="swf")

            # pt pieces: [0:1184]=mb+chunks0-7, then 1024-col pieces
            nc.sync.dma_start(out=spta[:, 0:1184], in_=pta[:, 0:1184])
            nc.scalar.dma_start(out=spta[:, 1184:2208], in_=pta[:, 1184:2208])
            nc.gpsimd.dma_start(out=spta[:, 2208:3232], in_=pta[:, 2208:3232])
            nc.gpsimd.dma_start(out=spta[:, 3232:4256], in_=pta[:, 3232:4256])
            # basis on SP in gen-consumption order (high cols first);
            # Act only gets small early loads so its recip window is clear
            nc.scalar.dma_start(out=scst, in_=cstb[:, :])
            nc.sync.dma_start(out=sgb[:, 9560:BSPLIT],
                              in_=gbb[:, 9560:BSPLIT])
            nc.scalar.dma_start(out=swf[:, 0:664], in_=wfb[:, 0:664])
            nc.gpsimd.dma_start(out=swf[:, 664:1688], in_=wfb[:, 664:1688])
            nc.sync.dma_start(out=sgb[:, 5460:7508], in_=gbb[:, 5460:7508])
            nc.sync.dma_start(out=sgb[:, 7508:9560], in_=gbb[:, 7508:9560])
            nc.gpsimd.dma_start(out=sgb[:, 0:2730], in_=gbb[:, 0:2730])
            nc.gpsimd.dma_start(out=sgb[:, 2730:5460], in_=gbb[:, 2730:5460])



            jnk = sb.tile([1, 4], f32, tag="jnk")
            se0 = nc.scalar
            se0.add_instruction(
                mybir.InstActivation(
                    name=nc.get_next_instruction_name(),
                    func=mybir.ActivationFunctionType.Reciprocal,
                    ins=[se0.lower_ap(spta[0:1, 0:4]),
                         mybir.ImmediateValue(dtype=f32, value=0.0),
                         mybir.ImmediateValue(dtype=f32, value=1.0),
                         mybir.ImmediateValue(dtype=f32, value=0.0)],
                    outs=[se0.lower_ap(jnk)],
                ))

            # ---- moments ----
            psmom = pmisc.tile([128, 8], f32, tag="m1")
            for n_, c in enumerate(range(32)):
                nc.tensor.matmul(
                    psmom[:, 0:5],
                    lhsT=spta[:, 160 + c * 128:160 + (c + 1) * 128],
                    rhs=spta[:, c * 5:(c + 1) * 5],
                    start=(n_ == 0),
                    stop=(n_ == 31),
                )


            # ---- coefficient chain, Pool (g) / DVE (v) split ----
            def t(cols, tag):
                return sb.tile([128, cols], f32, tag=tag, name=tag)

            v = nc.vector
            g = v
            mom = t(5, "mom")
            v.tensor_copy(out=mom, in_=psmom[:, 0:5])  # muy mux Eyy Eyx Exx
            pyy = t(1, "pyy"); pyx_ = t(1, "pyx_"); pxx = t(1, "pxx")
            g.tensor_tensor(out=pyy, in0=mom[:, 0:1], in1=mom[:, 0:1],
                            op=AT.mult)
            g.tensor_tensor(out=pyx_, in0=mom[:, 0:1], in1=mom[:, 1:2],
                            op=AT.mult)
            g.tensor_tensor(out=pxx, in0=mom[:, 1:2], in1=mom[:, 1:2],
                            op=AT.mult)
            c00 = t(1, "c00"); c01 = t(1, "c01"); c11 = t(1, "c11")
            g.tensor_tensor(out=c00, in0=mom[:, 2:3], in1=pyy, op=AT.subtract)
            g.tensor_tensor(out=c01, in0=mom[:, 3:4], in1=pyx_,
                            op=AT.subtract)
            g.tensor_tensor(out=c11, in0=mom[:, 4:5], in1=pxx, op=AT.subtract)
            pp = t(2, "pp")
            v.tensor_scalar(out=pp, in0=mom[:, 0:2], scalar1=-1.0,
                            scalar2=EPS_DIST, op0=AT.mult, op1=AT.add)
            qyy = t(1, "qyy"); qyx = t(1, "qyx"); qxx = t(1, "qxx")
            v.tensor_tensor(out=qyy, in0=pp[:, 0:1], in1=pp[:, 0:1],
                            op=AT.mult)
            v.tensor_tensor(out=qyx, in0=pp[:, 0:1], in1=pp[:, 1:2],
                            op=AT.mult)
            v.tensor_tensor(out=qxx, in0=pp[:, 1:2], in1=pp[:, 1:2],
                            op=AT.mult)
            m1_ = t(1, "m1_"); m2_ = t(1, "m2_")
            g.tensor_tensor(out=m1_, in0=c00, in1=c11, op=AT.mult)
            g.tensor_tensor(out=m2_, in0=c01, in1=c01, op=AT.mult)
            d2 = t(1, "d2")
            g.tensor_tensor(out=d2, in0=m1_, in1=m2_, op=AT.subtract)
            d2s = t(1, "d2s")
            g.tensor_scalar(out=d2s, in0=d2,
                            scalar1=1.0 / (L_INV_SCAL * L_INV_SCAL),
                            scalar2=1e-20, op0=AT.mult, op1=AT.add)
            q = t(1, "q")
            v.reciprocal_approx_fast(out=q, in_=d2s)

            coef = sb.tile([128, 6], f32, tag="coef")
            g.tensor_tensor(out=coef[:, 3:4], in0=q, in1=c11, op=AT.mult)
            g.scalar_tensor_tensor(out=coef[:, 4:5], in0=q, scalar=-2.0,
                                   in1=c01, op0=AT.mult, op1=AT.mult)
            g.tensor_tensor(out=coef[:, 5:6], in0=q, in1=c00, op=AT.mult)
            tA = t(1, "tA"); tB = t(1, "tB"); tC = t(1, "tC")
            v.tensor_tensor(out=tA, in0=coef[:, 3:4], in1=qyy, op=AT.mult)
            v.tensor_tensor(out=tB, in0=coef[:, 4:5], in1=qyx, op=AT.mult)
            v.tensor_tensor(out=tC, in0=coef[:, 5:6], in1=qxx, op=AT.mult)
            c0a = t(1, "c0a")
            v.tensor_tensor(out=c0a, in0=tA, in1=tB, op=AT.add)
            c0b = t(1, "c0b")
            v.tensor_tensor(out=c0b, in0=c0a, in1=tC, op=AT.add)
            v.tensor_scalar_add(out=coef[:, 0:1], in0=c0b, scalar1=1.0)
            t4 = t(1, "t4"); t5 = t(1, "t5")
            g.tensor_tensor(out=t4, in0=coef[:, 3:4], in1=pp[:, 0:1],
                            op=AT.mult)
            g.tensor_tensor(out=t5, in0=coef[:, 4:5], in1=pp[:, 1:2],
                            op=AT.mult)
            g.scalar_tensor_tensor(out=coef[:, 1:2], in0=t4, scalar=2.0,
                                   in1=t5, op0=AT.mult, op1=AT.add)
            t6 = t(1, "t6"); t7 = t(1, "t7")
            g.tensor_tensor(out=t6, in0=coef[:, 4:5], in1=pp[:, 0:1],
                            op=AT.mult)
            g.tensor_tensor(out=t7, in0=coef[:, 5:6], in1=pp[:, 1:2],
                            op=AT.mult)
            g.scalar_tensor_tensor(out=coef[:, 2:3], in0=t7, scalar=2.0,
                                   in1=t6, op0=AT.mult, op1=AT.add)

            pst = pmisc.tile([6, 128], f32, tag="m1")
            nc.tensor.transpose(pst, coef, scst[:, 0:128])
            coefT = sb.tile([6, 128], bf16, tag="coefT")
            nc.vector.tensor_copy(out=coefT, in_=pst)
            # band lhsT variants: A = [coefT; 0] (legal partition-32 copy),
            # B = [0; coefT] (SBUF->SBUF DMA; latency hides under band A)
            nc.vector.tensor_copy(out=czzA[0:6, :], in_=coefT)
            nc.sync.dma_start(out=czzB[6:12, :], in_=coefT)

            # ---- heat generation ----
            heat = sb.tile([128, GB_TOT], f32, tag="heat")

            def recip_dve(off, n, ps):
                nc.vector.reciprocal_approx_fast(
                    out=heat[:, off:off + n], in_=ps)

            def recip_act(off, n, ps):
                se = nc.scalar
                se.add_instruction(
                    mybir.InstActivation(
                        name=nc.get_next_instruction_name(),
                        func=mybir.ActivationFunctionType.Reciprocal,
                        ins=[se.lower_ap(ps),
                             mybir.ImmediateValue(dtype=f32, value=0.0),
                             mybir.ImmediateValue(dtype=f32, value=1.0),
                             mybir.ImmediateValue(dtype=f32, value=0.0)],
                        outs=[se.lower_ap(heat[:, off:off + n])],
                    ))

            # ---- fmap chains (bf16), fed from an Hb bf16 copy ----
            fstate = {}
            HB0 = int(HOFF[3])

            def fm_sel(si):
                hw = HWS[3 + si]
                hb0 = int(HOFF[3 + si]) - HB0
                pss = pmisc.tile([8, 256], f32, tag="m1")
                nc.tensor.matmul(pss[:, 0:hw],
                                 lhsT=swf[:, si * 8:si * 8 + 8],
                                 rhs=fstate["Hb"][:, hb0:hb0 + hw],
                                 start=True, stop=True)
                rt = sb.tile([8, 256], f32, tag="rt", bufs=2)
                nc.vector.tensor_scalar_add(out=rt[:, 0:hw], in0=pss[:, 0:hw],
                                            scalar1=1.0)
                rr = sb.tile([8, 256], f32, tag="rr", bufs=2)
                nc.vector.reciprocal_approx_fast(out=rr[:, 0:hw],
                                                 in_=rt[:, 0:hw])
                rrb = sb.tile([8, 256], bf16, tag="rrb", bufs=2)
                nc.vector.tensor_copy(out=rrb[:, 0:hw], in_=rr[:, 0:hw])
                fstate[si] = (rrb, hw)

            def fm_rep(si):
                rrb, hw = fstate[si]
                H = heat[:, int(HOFF[3 + si]):int(HOFF[3 + si]) + hw]
                psR = pmisc.tile([128, 256], f32, tag="m1")
                nc.tensor.matmul(psR[:, 0:hw], lhsT=swf[0:8, 24:152],
                                 rhs=rrb[:, 0:hw], start=True, stop=True)
                Hn = sb.tile([128, 256], bf16, tag="Hn", bufs=2)
                nc.vector.tensor_tensor(out=Hn[:, 0:hw], in0=H,
                                        in1=psR[:, 0:hw], op=AT.mult)
                fstate[si] = (Hn, hw)

            def fm_mm(si):
                Hn, hw = fstate[si]
                fm = sb.tile([128, 4 * hw], f32, tag=f"fm{si}")
                ntl = 2 if si == 0 else 1
                for tl in range(ntl):
                    psF = pmisc.tile([128, 512], f32, tag="psF")
                    for gi in range(4 // ntl):
                        gg = tl * (4 // ntl) + gi
                        nc.tensor.matmul(
                            psF[:, gi * hw:(gi + 1) * hw],
                            lhsT=swf[:, 152 + (si * 4 + gg) * 128:
                                     152 + (si * 4 + gg + 1) * 128],
                            rhs=Hn[:, 0:hw], start=True, stop=True)
                    cols = (4 // ntl) * hw
                    if si == 0 and tl == 0:
                        nc.vector.tensor_copy(out=fm[:, 0:cols],
                                              in_=psF[:, 0:cols])
                    else:
                        nc.scalar.copy(out=fm[:, tl * cols:(tl + 1) * cols],
                                       in_=psF[:, 0:cols])
                fstate[100 + si] = fm

            def hb_copy():
                Hb = sb.tile([128, 336], bf16, tag="Hb")
                nc.vector.tensor_copy(out=Hb, in_=heat[:, HOFF[3]:GB_TOT])
                fstate["Hb"] = Hb

            events = {
                1: lambda: hb_copy(),
                2: lambda: (fm_sel(0),
                            nc.gpsimd.dma_start(out=oreg(3),
                                                in_=heat[:, HOFF[3]:GB_TOT])),
                3: lambda: (fm_rep(0),
                            nc.gpsimd.dma_start(out=oreg(2),
                                                in_=heat[:, HOFF[2]:HOFF[3]])),
                4: lambda: fm_mm(0),
                5: lambda: fm_sel(1),
                6: lambda: fm_rep(1),
                7: lambda: fm_mm(1),
                8: lambda: fm_sel(2),
                9: lambda: fm_rep(2),
                10: lambda: fm_mm(2),
                11: lambda: (
                    nc.gpsimd.dma_start(out=oreg(4), in_=fstate[100]),
                    nc.gpsimd.dma_start(out=oreg(5), in_=fstate[101]),
                    nc.gpsimd.dma_start(out=oreg(1),
                                        in_=heat[:, HOFF[1]:HOFF[2]])),
                12: lambda: nc.gpsimd.dma_start(out=oreg(6), in_=fstate[102]),
            }

            ntile = len(GEN_TILES)
            for ti, (off, n, band) in enumerate(GEN_TILES):
                ps = pgen.tile([128, 512], f32, tag="ps")
                lhsT = czzA if band == 0 else czzB
                gc = off - BSPLIT if band == 0 else off
                nc.tensor.matmul(ps[:, 0:n], lhsT=lhsT,
                                 rhs=sgb[:, gc:gc + n],
                                 start=True, stop=True)
                if ti == ntile - 1:
                    recip_dve(off, n, ps[:, 0:n])
                elif ti == 40:
                    recip_act(off, n, ps[:, 0:n])
                else:
                    PAT11 = (recip_dve, recip_act, recip_dve, recip_act,
                             recip_dve, recip_act, recip_act, recip_dve,
                             recip_act, recip_dve, recip_act)
                    PAT11[ti % 11](off, n, ps[:, 0:n])
                if ti in events:
                    events[ti]()

            # stage-0 part heat: everything below heat col 16384 done last
            nc.sync.dma_start(out=oreg(0), in_=heat[:, HOFF[0]:HOFF[1]])
    nc.compile()
    return nc


def _get_nc():
    if "nc" not in _NC_CACHE:
        _NC_CACHE["nc"] = _build()
    return _NC_CACHE["nc"]


def _in_maps(part_maps, features):
    part_maps = np.asarray(part_maps, dtype=np.float32)
    features = np.asarray(features, dtype=np.float32)
    gb12, mb, cst, selb, repb = _host_consts()
    in_maps = []
    for core in range(NCORES):
        pm = part_maps[core * BL:(core + 1) * BL]
        pmr = pm.transpose(1, 0, 2, 3).reshape(ROWS, HMAP * HMAP)
        ptm = pmr.reshape(128, 32, 128).transpose(2, 1, 0).reshape(128, 4096)
        pta = np.concatenate([mb, ptm], axis=1).astype(np.float32)
        pta = np.ascontiguousarray(pta)
        wf = _host_wf(features[core * BL:(core + 1) * BL])
        wfp = np.concatenate([selb, repb, wf], axis=1)
        in_maps.append({"pta": pta, "gbb": gb12, "cstb": cst, "wfb": wfp})
    return in_maps


def _unpack(arr):
    """arr: [OUT_ROWS, STP] padded f32 -> (BL, OUT_TOT) for one core."""
    out = np.empty((BL, OUT_TOT), dtype=np.float32)

    def reg(ri, shape):
        r0, r1 = int(REGION_ROFF[ri]), int(REGION_ROFF[ri + 1])
        return arr[r0:r1, 0:REGION_W[ri]].reshape(shape)

    for s in (0, 1, 2):
        hw = HWS[s]
        block = reg(s, (NK, BL, hw))
        out[:, OUT_PH[s]:OUT_PH[s] + NK * hw] = \
            block.transpose(1, 0, 2).reshape(BL, NK * hw)
    b345 = reg(3, (128, 336))
    for s, c0 in ((3, 0), (4, 256), (5, 320)):
        pd = STAGES[s][2]
        hw = HWS[s]
        blk = b345[0:pd * BL, c0:c0 + hw].reshape(pd, BL, hw)
        out[:, OUT_PH[s]:OUT_PH[s] + pd * hw] = \
            blk.transpose(1, 0, 2).reshape(BL, pd * hw)
    for si, sidx in enumerate((3, 4, 5)):
        hw = HWS[sidx]
        block = reg(4 + si, (128, 4 * hw))
        fb = block.reshape(2, NF, 4, hw)             # (bo, n, g, f)
        fmap = fb.transpose(2, 0, 1, 3).reshape(BL, NF * hw)
        out[:, OUT_FM[sidx]:OUT_FM[sidx] + NF * hw] = fmap
    return out


def _run(part_maps, features, trace=False):
    from concourse.bass_utils import run_bass_kernel_spmd
    nc = _get_nc()
    res = run_bass_kernel_spmd(nc, _in_maps(part_maps, features),
                               list(range(NCORES)), trace=trace)
    outs = [_unpack(res.results[i]["out"]) for i in range(NCORES)]
    return np.concatenate(outs, axis=0), res


def kernel(part_maps, features):
    out, _ = _run(part_maps, features, trace=False)
    return out
